# revision 23
# baseline (speedup 1.0000x reference)
"""2-layer GCN (GCNConv x2) on 8 Trainium2 NeuronCores.

Strategy (dst-sharded, edge-partitioned by destination; static-count gathers):
- Each core owns N/8 destination nodes and the edges pointing at them
  (plus the GCN self-loops, kept out of the edge stream).
- Layer-1 table: h~ = (dinv*x) @ W1 computed per-shard (x host-prescaled by
  dinv), ONE AllGather into hfull; gather banks = 4 int16-addressable row
  slices of hfull (rank-pair layout).
- Layer-2 table: h2~ = dinv * h2 written per-quarter (block-aligned
  quarters), 4 bank-wise AllGathers fired as quarters complete so they
  overlap layer-1's gather phase; bank q = concat over cores of quarter q.
- Per (super-block of SB_N dst blocks, bank): edges packed contiguously in
  block order into 128-slot chunks (straddling block boundaries); one
  dma_gather per segment on queue=bank (4 SWDGE queue contexts).
  Scatter-add via is_equal-indicator matmuls accumulating in PSUM.
- Gather counts are STATIC: every core gathers the max budget per segment,
  with idx 0 padding (gathers row 0 harmlessly; dstloc=-1 zeroes the
  indicator).  No per-gather reg_loads -> no WAR serialization through a
  shared register; the Q7 descgen queues pipeline freely.  num_idxs_reg
  comes from a small pool of registers preloaded once per distinct size.
- Layer-2 bank-3 gathers are DELAYED by L2_K3_DELAY super-blocks in issue
  order so the head of the in-order GpSimd stream never waits on the last
  AG2 (whose input is only ready when layer 1 finishes).
- Layer 1 accumulates transposed (aggT [feat, dst]) so bias+ReLU ride the
  activation engine and out1 feeds h2 = out1 @ W2 directly as lhsT.
"""
import sys
import types

import numpy as np
import ml_dtypes

P = 128
NCORES = 8
GMAX = 32  # max chunks (128 idxs each) per dma_gather
SB_N = 3  # dst blocks per super-block
NQUEUES = 4
XGRP = 25
GBUFS = 10
SBUFS = 8
AG2_LAG = 2  # super-blocks between a quarter finishing and its AG2 trigger
L2_K3_DELAY = 1  # super-blocks by which layer-2 bank-3 gathers trail
# (PSUM is bank-granular: 8 banks of 2KB; psagg needs (delay+1)*SB_N <= 6)
G_SB = 2  # super-blocks merged per dma_gather (amortizes Q7 fixed cost)

_CACHE = {}


# ---------------------------------------------------------------- compat ---
def _install_compat():
    """Patches for this axon/walrus stack (drain waits, per-inst wait caps,
    NTFF shim). Idempotent."""
    if _CACHE.get("compat"):
        return
    import concourse.tile as tile
    import concourse.mybir as mybir

    _ev = [0]

    def _split_inst_waits(ordered):
        for _bb, insts in ordered.items():
            out = []
            for inst in insts:
                si = getattr(inst, "sync_info", None)
                if si is not None and si.on_wait is not None and len(si.on_wait) > 1:
                    waits = list(si.on_wait)
                    excess, keep = waits[:-1], waits[-1:]
                    si.on_wait.clear()
                    for sw in keep:
                        si.on_wait.append(sw)
                    for i in range(0, len(excess), 2):
                        _ev[0] += 1
                        ev = mybir.InstEventSemaphore(
                            name=f"evsplit-{_ev[0]}", ins=[], outs=[]
                        )
                        ev.engine = inst.engine
                        ev.sync_info = mybir.SyncInfo(
                            on_wait=excess[i : i + 2], on_update=[]
                        )
                        out.append(ev)
                out.append(inst)
            insts[:] = out

    orig_lower = tile.TileContext._lower_ordered_insts

    def patched_lower(self, ordered):
        _split_inst_waits(ordered)
        return orig_lower(self, ordered)

    def patched_drain(self, tick_clock, wait_clock):
        sems_alloc = list(self.sems.allocated().values())
        carrier = self.nc.sync.wait_ge(sems_alloc[0], 0)
        wait_clock.add_sem_waits(
            carrier.ins, tile.ScopedClock({None: tick_clock.global_clock})
        )
        waits = list(carrier.ins.sync_info.on_wait)
        carrier.ins.sync_info.on_wait.clear()
        for sw in waits[:2]:
            carrier.ins.sync_info.on_wait.append(sw)
        for i in range(2, len(waits), 2):
            c = self.nc.sync.wait_ge(sems_alloc[0], 0)
            c.ins.sync_info.on_wait.clear()
            for sw in waits[i : i + 2]:
                c.ins.sync_info.on_wait.append(sw)
        self.nc.sync.drain(fusable=False)
        self.nc.all_engine_barrier()
        popped = self.nc._tile_sem_poison_stack.pop()
        assert popped is self._sem_poison
        self.nc.clear_and_free_semaphores(sems_alloc)
        self.nc.all_engine_barrier()

    tile.TileContext._lower_ordered_insts = patched_lower
    tile.TileContext._drain_and_barrier = patched_drain

    # NTFF profile hook shim (missing antenv.axon_hooks in this image)
    _hook = {}
    mod = types.ModuleType("antenv.axon_hooks")
    mod.set_axon_ntff_profile_hook = lambda h: _hook.update(hook=h)
    mod.get_axon_ntff_profile_hook = lambda: _hook.get("hook")
    sys.modules["antenv.axon_hooks"] = mod
    try:
        import antenv

        antenv.axon_hooks = mod
        from trn_agent_boot.trn_boot import _ntff_profile_via_ctypes

        mod.set_axon_ntff_profile_hook(
            _ntff_profile_via_ctypes("/opt/axon/libaxon_pjrt.so")
        )
    except Exception:
        pass
    _CACHE["compat"] = True


# ---------------------------------------------------------- preprocessing ---
class Schedule:
    pass


class LayerSched:
    pass


def _quarter_bounds(nblk):
    base, rem = nblk // 4, nblk % 4
    sizes = [base + (1 if i < rem else 0) for i in range(4)]
    starts = np.cumsum([0] + sizes)
    return [(int(starts[i]), int(starts[i + 1])) for i in range(4)]


def _make_layer_sched(
    n, nblk, n_sb, s_core, s_pair, s_bank, s_blk, s_bidx, s_dstloc
):
    """Build the per-(super-block-pair, bank) straddle-packed schedule for
    one bank mapping.  Inputs are edge arrays sorted by (core, pair, bank,
    blk).  Gather counts are static: all cores gather the full per-segment
    budget, with zero-index padding (row 0) beyond their real edges.
    Each gather's slots are split into per-super-block groups so indicator
    tiles and PE matmuls stay super-block granular."""
    e = s_core.shape[0]
    npair = (n_sb + G_SB - 1) // G_SB
    key = ((s_core * npair + s_pair) * 4 + s_bank) * nblk + s_blk
    cnt = np.bincount(key, minlength=NCORES * npair * 4 * nblk).reshape(
        NCORES, npair, 4, nblk
    )

    gathers = []
    slots = []
    chunk_gid = 0
    budget_tab = np.zeros((npair, 4), np.int64)
    for p in range(npair):
        blocks = list(
            range(p * G_SB * SB_N, min((p + 1) * G_SB * SB_N, nblk))
        )
        for k in range(4):
            percore = cnt[:, p, k, :][:, blocks]
            cum = np.cumsum(percore, axis=1)
            budget = max(int(np.ceil(cum[:, -1].max() / P)), 1)
            budget_tab[p, k] = budget
            lo = np.min(cum - percore, axis=0)
            hi = np.max(cum, axis=0)
            g0 = 0
            while g0 < budget:
                gn = min(GMAX, budget - g0)
                gi = len(gathers)
                gsl = []
                for j in range(g0, g0 + gn):
                    c_lo, c_hi = j * P, (j + 1) * P
                    for bi, b in enumerate(blocks):
                        if hi[bi] > c_lo and lo[bi] < c_hi:
                            gsl.append(
                                dict(
                                    g=gi,
                                    cl=j - g0,
                                    blk=b,
                                    sb=b // SB_N,
                                    bank=k,
                                    chunk_gid=chunk_gid + j,
                                )
                            )
                gathers.append(
                    dict(
                        gi=gi,
                        pair=p,
                        bank=k,
                        chunk0=chunk_gid + g0,
                        nch=gn,
                        nidx=gn * P,
                        _gsl=gsl,
                        c16=(chunk_gid + g0) * P // 16,
                    )
                )
                g0 += gn
            chunk_gid += budget
    totc = chunk_gid
    tot_slots = totc * P

    # assign global slot ids grouped by (gather, sb): dstloc column order is
    # host-chosen, so per-(gather, sb) indicator slices stay contiguous even
    # though chunk block-ranges interleave super-blocks across cores.
    first_slot_of_block = {}
    last_slot_of_block = {}
    for g in gathers:
        gsl = g.pop("_gsl")
        groups = {}
        for sb_val in sorted({sl["sb"] for sl in gsl}):
            grp_slots = [sl for sl in gsl if sl["sb"] == sb_val]
            grp_start = len(slots)
            for i, sl in enumerate(grp_slots):
                sl["slot_gid"] = len(slots)
                sl["sl_in_grp"] = i
                first_slot_of_block.setdefault((sb_val, sl["blk"]), len(slots))
                last_slot_of_block[(sb_val, sl["blk"])] = len(slots)
                slots.append(sl)
            groups[sb_val] = [grp_start, len(slots), grp_slots]
        g["groups"] = groups
    nslots = len(slots)

    # the self-loop identity matmul opens each block's PSUM group (start);
    # the last slot closes it (stop).  Slot-id order matches PE emission
    # order (pair, bank, sb-group, chunk), so the max id per block is the
    # last matmul emitted for it.
    for i, sl in enumerate(slots):
        sl["start"] = False
        sl["stop"] = last_slot_of_block[(sl["sb"], sl["blk"])] == i
    has_slots = set(first_slot_of_block.keys())

    seg_key = (s_core * npair + s_pair) * 4 + s_bank
    seg_ptr = np.searchsorted(seg_key, np.arange(NCORES * npair * 4 + 1))
    seg_chunk0 = {}
    cg = 0
    for p in range(npair):
        for k in range(4):
            seg_chunk0[(p, k)] = cg
            cg += int(budget_tab[p, k])

    # idx streams: zero padding beyond each core's real edges (gathers row 0;
    # dstloc stays -1 so the indicator kills the contribution).  Counts are
    # therefore identical across cores -> no dynamic trim registers.
    idx_flat = np.zeros((NCORES, tot_slots), np.int16)
    for c in range(NCORES):
        arr = idx_flat[c]
        for p in range(npair):
            for k in range(4):
                p0 = seg_ptr[(c * npair + p) * 4 + k]
                p1 = seg_ptr[(c * npair + p) * 4 + k + 1]
                base = seg_chunk0[(p, k)] * P
                arr[base : base + p1 - p0] = s_bidx[p0:p1].astype(np.int16)

    dstloc_s = np.full((NCORES, P, nslots), -1.0, np.float32)
    seg_id = (s_core * npair + s_pair) * 4 + s_bank
    pos_in_seg = np.arange(e) - seg_ptr[seg_id]
    seg_chunk0_arr = np.zeros(NCORES * npair * 4, np.int64)
    for p in range(npair):
        for k in range(4):
            for c in range(NCORES):
                seg_chunk0_arr[(c * npair + p) * 4 + k] = seg_chunk0[(p, k)]
    chunk_of_edge = seg_chunk0_arr[seg_id] + pos_in_seg // P
    part_of_edge = pos_in_seg % P
    slot_lut = np.full((totc, nblk), -1, np.int64)
    for i, sl in enumerate(slots):
        slot_lut[sl["chunk_gid"], sl["blk"]] = i
    slot_of_edge = slot_lut[chunk_of_edge, s_blk]
    assert (slot_of_edge >= 0).all()
    dstloc_s[s_core, part_of_edge, slot_of_edge] = s_dstloc

    idx_stream = np.ascontiguousarray(
        idx_flat.reshape(NCORES, tot_slots // 16, 16).transpose(0, 2, 1)
    )
    idx_stream = np.tile(idx_stream, (1, 8, 1))

    ls = LayerSched()
    ls.npair = npair
    ls.has_slots = has_slots
    ls.gathers = gathers
    ls.slots = slots
    ls.totc, ls.nslots, ls.tot_slots = totc, nslots, tot_slots
    # max per-(gather, sb) group size -> indicator tile width
    ls.grpmax = max(
        (grp[1] - grp[0] for g in gathers for grp in g["groups"].values()),
        default=1,
    )
    ls.gchmax = max(g["nch"] for g in gathers)
    ls.idx_stream = idx_stream
    ls.idx_flat = idx_flat
    ls.dstloc_s = dstloc_s.astype(ml_dtypes.bfloat16)
    return ls


def _preprocess(n, edge_index):
    src0 = np.asarray(edge_index[0], np.int64)
    dst0 = np.asarray(edge_index[1], np.int64)
    shard = n // NCORES
    nblk = (shard + P - 1) // P
    n_sb = (nblk + SB_N - 1) // SB_N
    qb = _quarter_bounds(nblk)
    qrow_start = [b0 * P for b0, b1 in qb]
    qrows = [min(b1 * P, shard) - b0 * P for b0, b1 in qb]
    bank2_n = [NCORES * r for r in qrows]
    assert all(b <= 32767 for b in bank2_n)
    bank1_rows = (n + 3) // 4  # rank-pair banks for layer 1 (slices of hfull)
    assert bank1_rows <= 32767

    deg = np.bincount(dst0, minlength=n).astype(np.float64) + 1.0
    dinv = (1.0 / np.sqrt(deg)).astype(np.float32)

    # self-loops are NOT in the edge stream: both layers fold them in with
    # one identity matmul per block from the per-core hloc/h2loc tensors.
    src = src0
    dst = dst0

    core_d = dst // shard
    dl = dst - core_d * shard
    blk = dl // P
    dstloc = (dl % P).astype(np.int64)
    sb = blk // SB_N

    # layer-1 bank mapping: contiguous row slices of hfull (rank-major)
    bank_a = src // bank1_rows
    bidx_a = src - bank_a * bank1_rows

    # layer-2 bank mapping: quarter-stacked
    core_s = src // shard
    off = src - core_s * shard
    sblk = off // P
    qb_arr = np.zeros(nblk, np.int64)
    for q, (b0, b1) in enumerate(qb):
        qb_arr[b0:b1] = q
    bank_b = qb_arr[sblk]
    bidx_b = core_s * np.array(qrows)[bank_b] + (off - np.array(qrow_start)[bank_b])

    pair = sb // G_SB
    scheds = []
    for bank, bidx in ((bank_a, bidx_a), (bank_b, bidx_b)):
        order = np.lexsort((blk, bank, pair, core_d))
        scheds.append(
            _make_layer_sched(
                n,
                nblk,
                n_sb,
                core_d[order],
                pair[order],
                bank[order],
                blk[order],
                bidx[order],
                dstloc[order],
            )
        )

    sch = Schedule()
    sch.n, sch.shard, sch.nblk, sch.n_sb = n, shard, nblk, n_sb
    sch.e = src.shape[0]
    sch.qb, sch.qrow_start, sch.qrows = qb, qrow_start, qrows
    sch.bank1_rows, sch.bank2_n = bank1_rows, bank2_n
    sch.dinv = dinv
    sch.L1, sch.L2 = scheds
    sch.grpmax = max(sch.L1.grpmax, sch.L2.grpmax)
    sch.gchmax = max(sch.L1.gchmax, sch.L2.gchmax)
    sch.nidx_vals = sorted(
        {g["nidx"] for g in sch.L1.gathers} | {g["nidx"] for g in sch.L2.gathers}
    )
    return sch


# ----------------------------------------------------------------- build ---
def _build(sch, in_dim, hid, out_dim):
    import concourse.mybir as mybir
    import concourse.tile as tile
    from concourse import bacc

    bf16 = mybir.dt.bfloat16
    f32 = mybir.dt.float32
    shard, nblk, n_sb = sch.shard, sch.nblk, sch.n_sb
    grpmax = sch.grpmax
    gchmax = sch.gchmax
    qb = sch.qb
    n = sch.n

    nc = bacc.Bacc(num_swdge_queues=NQUEUES)

    xT = nc.declare_dram_parameter("xT", [in_dim, shard], bf16, isOutput=False)
    idxs1 = nc.declare_dram_parameter(
        "idxs1", [P, sch.L1.tot_slots // 16], mybir.dt.int16, isOutput=False
    )
    idxs2 = nc.declare_dram_parameter(
        "idxs2", [P, sch.L2.tot_slots // 16], mybir.dt.int16, isOutput=False
    )
    dstloc1 = nc.declare_dram_parameter(
        "dstloc1", [P, sch.L1.nslots], bf16, isOutput=False
    )
    dstloc2 = nc.declare_dram_parameter(
        "dstloc2", [P, sch.L2.nslots], bf16, isOutput=False
    )
    iotar_in = nc.declare_dram_parameter("iotar", [P, grpmax * P], bf16, isOutput=False)
    dinvbc = nc.declare_dram_parameter("dinvbc", [P, nblk * P], f32, isOutput=False)
    dinvb = nc.declare_dram_parameter("dinvb", [P, nblk], f32, isOutput=False)
    w1 = nc.declare_dram_parameter("W1", [in_dim, hid], bf16, isOutput=False)
    b1 = nc.declare_dram_parameter("b1", [hid, 1], f32, isOutput=False)
    w2 = nc.declare_dram_parameter("W2", [hid, out_dim], bf16, isOutput=False)
    b2bc = nc.declare_dram_parameter("b2bc", [P, out_dim], f32, isOutput=False)
    ident_in = nc.declare_dram_parameter("ident", [P, P], bf16, isOutput=False)
    out_ext = nc.declare_dram_parameter("out", [shard, out_dim], f32, isOutput=True)

    dummy_tbl = nc.dram_tensor("dummy_tbl", [1, P], bf16)
    hloc = nc.dram_tensor("hloc", [shard, P], bf16)
    hfull = nc.dram_tensor("hfull", [n, P], bf16, addr_space="Shared")
    h2loc_q = [
        nc.dram_tensor(f"h2loc{q}", [sch.qrows[q], P], bf16) for q in range(4)
    ]
    h2bank = [
        nc.dram_tensor(f"h2bank{q}", [sch.bank2_n[q], P], bf16, addr_space="Shared")
        for q in range(4)
    ]

    kin = in_dim // P

    def quarter_of(b):
        for q, (b0, b1) in enumerate(qb):
            if b0 <= b < b1:
                return q
        raise AssertionError

    def layer_maps(ls):
        seg_gathers = {}
        for g in ls.gathers:
            seg_gathers.setdefault((g["pair"], g["bank"]), []).append(g)
        return seg_gathers

    seg1 = layer_maps(sch.L1)
    seg2 = layer_maps(sch.L2)

    ag2_at_sb = {}
    for q in range(4):
        sq_end = (qb[q][1] - 1) // SB_N
        key = min(sq_end + AG2_LAG, n_sb - 1) if q < 3 else n_sb - 1
        ag2_at_sb.setdefault(key, []).append(q)

    with tile.TileContext(nc) as tc:
        with (
            tc.tile_pool(name="const", bufs=1) as cpool,
            tc.tile_pool(name="xload", bufs=2) as xpool,
            tc.tile_pool(name="hb", bufs=3) as hbpool,
            tc.tile_pool(name="idx", bufs=16) as ipool,
            tc.tile_pool(name="gath", bufs=GBUFS) as gpool,
            tc.tile_pool(name="sind", bufs=SBUFS) as spool,
            tc.tile_pool(name="dvp", bufs=3) as dvpool,
            tc.tile_pool(name="rl", bufs=8) as rpool,
            tc.tile_pool(name="blk", bufs=3) as bpool,
            tc.tile_pool(name="ob", bufs=2) as opool,
            tc.tile_pool(name="psh", bufs=1, space="PSUM") as psh,
            tc.tile_pool(
                name="psagg", bufs=(L2_K3_DELAY + 1) * SB_N, space="PSUM"
            ) as psagg,
            tc.tile_pool(name="psh2", bufs=1, space="PSUM") as psh2,
        ):
            import contextlib

            regstack = contextlib.ExitStack()
            # one register per distinct static gather size, loaded once --
            # no per-gather register traffic, no WAR serialization.
            nidx_regs = {}
            for v in sch.nidx_vals:
                r = regstack.enter_context(nc.gpsimd.register(f"nidx_{v}"))
                nc.gpsimd.reg_mov(r, v)
                nidx_regs[v] = r

            # warm-up gathers: one tiny gather per register so every size
            # register is READ at t~0 -- otherwise Tile places a register's
            # MOVE just before its first (possibly late) consumer with
            # conservative waits on the collectives, stalling the stream.
            zit = ipool.tile([P, gchmax * 8], mybir.dt.int16, tag="it", name="zit")
            nc.vector.memset(zit[:], 0.0)
            for wi, v in enumerate(sch.nidx_vals):
                gwt = gpool.tile([P, gchmax, P], bf16, tag="gt", name=f"warm{wi}")
                nc.gpsimd.dma_gather(
                    out_ap=gwt[:, : v // P, :],
                    in_ap=dummy_tbl[0:1, :],
                    idxs_ap=zit[:, : v // 16],
                    num_idxs=v,
                    num_idxs_reg=nidx_regs[v],
                    elem_size=P,
                    single_packet=False,
                    queue_num=wi % NQUEUES,
                )

            # ---- constants into SBUF
            w1_t = [
                cpool.tile([P, hid], bf16, tag=f"w1_{k}", name=f"w1t{k}")
                for k in range(kin)
            ]
            for k in range(kin):
                nc.sync.dma_start(out=w1_t[k][:], in_=w1[k * P : (k + 1) * P, :])
            w2_sb = cpool.tile([hid, out_dim], bf16, tag="w2")
            nc.sync.dma_start(out=w2_sb[:], in_=w2[:])
            b1_sb = cpool.tile([hid, 1], f32, tag="b1")
            nc.sync.dma_start(out=b1_sb[:], in_=b1[:])
            b2_sb = cpool.tile([P, out_dim], f32, tag="b2")
            nc.sync.dma_start(out=b2_sb[:], in_=b2bc[:])
            dinvb_sb = cpool.tile([P, nblk], f32, tag="dinvb")
            nc.sync.dma_start(out=dinvb_sb[:], in_=dinvb[:])
            dstloc1_sb = cpool.tile([P, sch.L1.nslots], bf16, tag="dstloc1")
            nc.sync.dma_start(out=dstloc1_sb[:], in_=dstloc1[:])
            dstloc2_sb = cpool.tile([P, sch.L2.nslots], bf16, tag="dstloc2")
            nc.sync.dma_start(out=dstloc2_sb[:], in_=dstloc2[:])
            iotar_sb = cpool.tile([P, grpmax * P], bf16, tag="iotar")
            nc.sync.dma_start(out=iotar_sb[:], in_=iotar_in[:])
            ident_sb = cpool.tile([P, P], bf16, tag="ident")
            nc.sync.dma_start(out=ident_sb[:], in_=ident_in[:])

            # ---- h~ = (dinv*x) @ W1, shard-local (x pre-scaled by dinv);
            # 4 blocks batched per PSUM bank, double-buffered
            for g0 in range(0, nblk, XGRP):
                g1 = min(g0 + XGRP, nblk)
                c0, c1 = g0 * P, min(g1 * P, shard)
                xt = [
                    xpool.tile([P, XGRP * P], bf16, tag=f"xt{k}", name=f"xt{k}")
                    for k in range(kin)
                ]
                for k in range(kin):
                    nc.sync.dma_start(
                        out=xt[k][:, : c1 - c0], in_=xT[k * P : (k + 1) * P, c0:c1]
                    )
                for s0 in range(g0, g1, 4):
                    s1 = min(s0 + 4, g1)
                    hp = psh.tile([P, 4 * hid], f32, tag="hps")
                    mtot = min(s1 * P, shard) - s0 * P
                    for b in range(s0, s1):
                        m = min(P, shard - b * P)
                        sub = b - s0
                        for k in range(kin):
                            nc.tensor.matmul(
                                out=hp[:m, sub * hid : (sub + 1) * hid],
                                lhsT=xt[k][:, b * P - c0 : b * P - c0 + m],
                                rhs=w1_t[k][:],
                                start=(k == 0),
                                stop=(k == kin - 1),
                            )
                    nsub = s1 - s0
                    hsb = hbpool.tile([P, 4, hid], bf16, tag="hsb")
                    nc.scalar.activation(
                        out=hsb[:, :nsub, :],
                        in_=hp[:, : nsub * hid].rearrange("p (g f) -> p g f", g=nsub),
                        func=mybir.ActivationFunctionType.Copy,
                    )
                    nfull = mtot // P
                    if nfull:
                        nc.sync.dma_start(
                            out=hloc[s0 * P : s0 * P + nfull * P, :].rearrange(
                                "(g p) f -> p g f", p=P
                            ),
                            in_=hsb[:, :nfull, :],
                        )
                    if mtot > nfull * P:
                        mp = mtot - nfull * P
                        nc.sync.dma_start(
                            out=hloc[s0 * P + nfull * P : s0 * P + mtot, :],
                            in_=hsb[:mp, nfull, :],
                        )

            nc.gpsimd.collective_compute(
                "AllGather",
                mybir.AluOpType.bypass,
                ins=[hloc[:]],
                outs=[hfull[:]],
                replica_groups=[list(range(NCORES))],
            )

            def bank_table(layer, k):
                if layer == 1:
                    r0 = k * sch.bank1_rows
                    r1 = min(r0 + sch.bank1_rows, n)
                    return hfull[r0:r1, :]
                return h2bank[k][0 : sch.bank2_n[k], :]

            def issue_gather(layer, g, it):
                nidx, nch = g["nidx"], g["nch"]
                gt = gpool.tile([P, gchmax, P], bf16, tag="gt")
                nc.gpsimd.dma_gather(
                    out_ap=gt[:, :nch, :],
                    in_ap=bank_table(layer, g["bank"]),
                    idxs_ap=it[:, : nidx // 16],
                    num_idxs=nidx,
                    num_idxs_reg=nidx_regs[nidx],
                    elem_size=P,
                    single_packet=False,
                    queue_num=g["bank"],
                )
                return gt

            def load_it(layer, g):
                idxs = idxs1 if layer == 1 else idxs2
                it = ipool.tile([P, gchmax * 8], mybir.dt.int16, tag="it")
                nc.sync.dma_start(
                    out=it[:, : g["nidx"] // 16],
                    in_=idxs[:, g["c16"] : g["c16"] + g["nidx"] // 16],
                )
                return it

            def build_ind(grp_start, grp_end, dstloc_sb):
                nsl = grp_end - grp_start
                sbig = spool.tile([P, grpmax, P], bf16, tag="sind")
                nc.vector.tensor_tensor(
                    out=sbig[:, :nsl, :],
                    in0=iotar_sb[:, : nsl * P].rearrange("p (k f) -> p k f", k=nsl),
                    in1=dstloc_sb[:, grp_start:grp_end].to_broadcast([P, nsl, P]),
                    op=mybir.AluOpType.is_equal,
                )
                return sbig

            def run_layer(layer):
                """Gathers are merged per (super-block pair, bank); slot
                matmuls and epilogues stay per super-block.  Layer 2 delays
                bank-3 gathers by one step so the in-order GpSimd stream
                never parks on the last AG2."""
                seg_gathers = seg1 if layer == 1 else seg2
                ls = sch.L1 if layer == 1 else sch.L2
                dstloc_sb = dstloc1_sb if layer == 1 else dstloc2_sb
                delay = 0 if layer == 1 else L2_K3_DELAY
                w = P if layer == 1 else out_dim

                def issued_at(t):
                    """gathers whose dma_gather is issued at step t."""
                    out = []
                    if t < n_sb and t % G_SB == 0:
                        p = t // G_SB
                        ks = range(4) if layer == 1 else range(3)
                        for k in ks:
                            out.extend(seg_gathers.get((p, k), []))
                    if layer == 2 and 0 <= t - 1 < n_sb and (t - 1) % G_SB == 0:
                        p = (t - 1) // G_SB
                        out.extend(seg_gathers.get((p, 3), []))
                    return out

                def slot_groups_at(t):
                    """(gather, sb) slot groups whose matmuls run at step t."""
                    out = []
                    if t < n_sb:
                        p = t // G_SB
                        ks = range(4) if layer == 1 else range(3)
                        for k in ks:
                            for g in seg_gathers.get((p, k), []):
                                if t in g["groups"]:
                                    out.append((g, t))
                    if layer == 2 and 0 <= t - 1 < n_sb:
                        p = (t - 1) // G_SB
                        for g in seg_gathers.get((p, 3), []):
                            if t - 1 in g["groups"]:
                                out.append((g, t - 1))
                    return out

                def prefetch_it(t):
                    return {g["gi"]: load_it(layer, g) for g in issued_at(t)}

                def prefetch_ind(t):
                    tiles = {}
                    for g, s in slot_groups_at(t):
                        grp = g["groups"][s]
                        tiles[(g["gi"], s)] = build_ind(grp[0], grp[1], dstloc_sb)
                    return tiles

                def prefetch_rl(s):
                    tiles = {}
                    if not (0 <= s < n_sb):
                        return tiles
                    blocks = list(range(s * SB_N, min((s + 1) * SB_N, nblk)))
                    # batch contiguous full blocks within one source tensor
                    runs = []
                    for b in blocks:
                        q = quarter_of(b) if layer == 2 else 0
                        if runs and runs[-1][2] == q and runs[-1][1] == b:
                            runs[-1][1] = b + 1
                        else:
                            runs.append([b, b + 1, q])
                    for b0, b1, q in runs:
                        nb = b1 - b0
                        m_end = min(b1 * P, shard) - b0 * P
                        rl = rpool.tile([P, SB_N, P], bf16, tag="rl")
                        src = hloc if layer == 1 else h2loc_q[q]
                        r0 = b0 * P - (0 if layer == 1 else sch.qrow_start[q])
                        nfull = m_end // P
                        if nfull:
                            nc.sync.dma_start(
                                out=rl[:, :nfull, :],
                                in_=src[r0 : r0 + nfull * P, :].rearrange(
                                    "(g p) f -> p g f", p=P
                                ),
                            )
                        if m_end > nfull * P:
                            mp = m_end - nfull * P
                            nc.sync.dma_start(
                                out=rl[:mp, nfull, :],
                                in_=src[r0 + nfull * P : r0 + m_end, :],
                            )
                        for b in range(b0, b1):
                            tiles[b] = (rl, b - b0)
                    return tiles

                def prefetch_dv(s):
                    if layer != 1 or not (0 <= s < n_sb):
                        return None
                    b0 = s * SB_N
                    b1 = min(b0 + SB_N, nblk)
                    dv = dvpool.tile([P, SB_N * P], f32, tag="dv")
                    nc.sync.dma_start(
                        out=dv[:, : (b1 - b0) * P], in_=dinvbc[:, b0 * P : b1 * P]
                    )
                    return dv

                it_tiles = prefetch_it(0)
                ind_tiles = prefetch_ind(0)
                rl_tiles = prefetch_rl(0)
                dv_cur = prefetch_dv(0)
                rl_hold = {}
                dv_hold = {}
                agg_hold = {}
                gt_tiles = {}
                nsteps = n_sb + delay
                for t in range(nsteps):
                    s_new = t if t < n_sb else None
                    s_old = t - delay if 0 <= t - delay < n_sb else None
                    next_it = prefetch_it(t + 1)
                    next_ind = prefetch_ind(t + 1)
                    next_rl = prefetch_rl(t + 1)
                    next_dv = prefetch_dv(t + 1)

                    if s_new is not None:
                        blocks = list(
                            range(s_new * SB_N, min((s_new + 1) * SB_N, nblk))
                        )
                        agg_t = {
                            b: psagg.tile(
                                [P, w], f32, tag="agg", name=f"agg{layer}_{s_new}_{b}"
                            )
                            for b in blocks
                        }
                        agg_hold[s_new] = agg_t
                        rl_hold[s_new] = rl_tiles
                        dv_hold[s_new] = dv_cur
                        # self-loop contribution opens each block's PSUM group
                        for b in blocks:
                            m = min(P, shard - b * P)
                            rl, sub = rl_tiles[b]
                            solo = (s_new, b) not in ls.has_slots
                            if layer == 1:
                                nc.tensor.matmul(
                                    out=agg_t[b][:, :],
                                    lhsT=rl[:m, sub, :],
                                    rhs=ident_sb[:m, :],
                                    start=True,
                                    stop=solo,
                                )
                            else:
                                nc.tensor.matmul(
                                    out=agg_t[b][:, :],
                                    lhsT=ident_sb[:m, :],
                                    rhs=rl[:m, sub, :out_dim],
                                    start=True,
                                    stop=solo,
                                )

                    for g in issued_at(t):
                        gt_tiles[g["gi"]] = issue_gather(
                            layer, g, it_tiles[g["gi"]]
                        )

                    for g, s in slot_groups_at(t):
                        gt = gt_tiles[g["gi"]]
                        sbig = ind_tiles[(g["gi"], s)]
                        agg_t = agg_hold[s]
                        for sl in g["groups"][s][2]:
                            if layer == 1:
                                nc.tensor.matmul(
                                    out=agg_t[sl["blk"]][:, :],
                                    lhsT=gt[:, sl["cl"], :],
                                    rhs=sbig[:, sl["sl_in_grp"], :],
                                    start=sl["start"],
                                    stop=sl["stop"],
                                )
                            else:
                                nc.tensor.matmul(
                                    out=agg_t[sl["blk"]][:, :],
                                    lhsT=sbig[:, sl["sl_in_grp"], :],
                                    rhs=gt[:, sl["cl"], :out_dim],
                                    start=sl["start"],
                                    stop=sl["stop"],
                                )

                    # ---- block epilogues for the super-block closing now
                    if s_old is not None:
                        blocks = list(
                            range(s_old * SB_N, min((s_old + 1) * SB_N, nblk))
                        )
                        agg_t = agg_hold.pop(s_old)
                        dv_sb = dv_hold.pop(s_old)
                        rl_hold.pop(s_old)
                        ob = (
                            opool.tile([P, SB_N, out_dim], f32, tag="ob", name="ob")
                            if layer == 2
                            else None
                        )
                        for b in blocks:
                            m = min(P, shard - b * P)
                            off = (b - s_old * SB_N) * P
                            if layer == 1:
                                t1 = bpool.tile([P, P], bf16, tag="t1")
                                nc.vector.tensor_tensor(
                                    out=t1[:],
                                    in0=agg_t[b][:, :],
                                    in1=dv_sb[:, off : off + P],
                                    op=mybir.AluOpType.mult,
                                )
                                o1 = bpool.tile([P, P], bf16, tag="o1")
                                nc.scalar.activation(
                                    out=o1[:],
                                    in_=t1[:],
                                    func=mybir.ActivationFunctionType.Relu,
                                    bias=b1_sb[:, :1],
                                )
                                h2p = psh2.tile([P, out_dim], f32, tag="h2p")
                                nc.tensor.matmul(
                                    out=h2p[:],
                                    lhsT=o1[:],
                                    rhs=w2_sb[:],
                                    start=True,
                                    stop=True,
                                )
                                h2s = bpool.tile([P, P], bf16, tag="h2s")
                                nc.scalar.activation(
                                    out=h2s[:m, :out_dim],
                                    in_=h2p[:m, :],
                                    func=mybir.ActivationFunctionType.Copy,
                                    scale=dinvb_sb[:m, b : b + 1],
                                )
                                q = quarter_of(b)
                                r0 = b * P - sch.qrow_start[q]
                                nc.sync.dma_start(
                                    out=h2loc_q[q][r0 : r0 + m, 0:out_dim],
                                    in_=h2s[:m, :out_dim],
                                )
                            else:
                                sub = b - blocks[0]
                                t2 = bpool.tile([P, out_dim], f32, tag="t2")
                                nc.scalar.activation(
                                    out=t2[:m, :],
                                    in_=agg_t[b][:m, :],
                                    func=mybir.ActivationFunctionType.Copy,
                                    scale=dinvb_sb[:m, b : b + 1],
                                )
                                nc.vector.tensor_tensor(
                                    out=ob[:m, sub, :],
                                    in0=t2[:m, :],
                                    in1=b2_sb[:m, :],
                                    op=mybir.AluOpType.add,
                                )
                        if layer == 2:
                            b0 = blocks[0]
                            m_end = min(blocks[-1] * P + P, shard) - b0 * P
                            nfull = m_end // P
                            if nfull:
                                nc.sync.dma_start(
                                    out=out_ext[
                                        b0 * P : b0 * P + nfull * P, :
                                    ].rearrange("(g p) f -> p g f", p=P),
                                    in_=ob[:, :nfull, :],
                                )
                            if m_end > nfull * P:
                                mp = m_end - nfull * P
                                nc.sync.dma_start(
                                    out=out_ext[b0 * P + nfull * P : b0 * P + m_end, :],
                                    in_=ob[:mp, nfull, :],
                                )

                    if layer == 1 and s_old is not None:
                        for q in ag2_at_sb.get(s_old, []):
                            nc.gpsimd.collective_compute(
                                "AllGather",
                                mybir.AluOpType.bypass,
                                ins=[h2loc_q[q][:]],
                                outs=[h2bank[q][:]],
                                replica_groups=[list(range(NCORES))],
                            )

                    it_tiles = next_it
                    ind_tiles = next_ind
                    rl_tiles = next_rl
                    dv_cur = next_dv

            run_layer(1)
            run_layer(2)
            regstack.close()

    nc.compile()
    return nc


# ---------------------------------------------------------------- kernel ---
def _make_in_maps(sch, x, W1, b1v, W2, b2v):
    hid = W1.shape[1]
    out_dim = W2.shape[1]
    shard, nblk = sch.shard, sch.nblk
    bf = ml_dtypes.bfloat16
    in_maps = []
    w1b = W1.astype(bf)
    w2b = W2.astype(bf)
    b1c = b1v.reshape(hid, 1).astype(np.float32).copy()
    b2c = np.broadcast_to(b2v.astype(np.float32), (P, out_dim)).copy()
    iotar = np.tile(np.arange(P, dtype=np.float32), (P, sch.grpmax)).astype(bf)
    ident = np.eye(P, dtype=np.float32).astype(bf)
    xs_all = (x * sch.dinv[:, None]).astype(bf)
    for c in range(NCORES):
        xs = np.ascontiguousarray(xs_all[c * shard : (c + 1) * shard].T)
        dv = sch.dinv[c * shard : (c + 1) * shard]
        full = np.zeros(nblk * P, np.float32)
        full[:shard] = dv
        dvb = np.ascontiguousarray(full.reshape(nblk, P).T)
        dbc = np.broadcast_to(full, (P, nblk * P)).copy()
        in_maps.append(
            {
                "xT": xs,
                "idxs1": sch.L1.idx_stream[c],
                "idxs2": sch.L2.idx_stream[c],
                "dstloc1": sch.L1.dstloc_s[c],
                "dstloc2": sch.L2.dstloc_s[c],
                "dinvb": dvb,
                "W1": w1b,
                "b1": b1c,
                "W2": w2b,
                "b2bc": b2c,
                "iotar": iotar,
                "ident": ident,
                "dinvbc": dbc,
            }
        )
    return in_maps


def _get_compiled(n, e, edge_index, in_dim, hid, out_dim):
    key = ("nc", n, e)
    if key not in _CACHE:
        sch = _preprocess(n, edge_index)
        _CACHE[("sched", n, e)] = sch
        _CACHE[key] = _build(sch, in_dim, hid, out_dim)
    return _CACHE[("sched", n, e)], _CACHE[key]


def kernel(x, edge_index, W1, b1, W2, b2):
    _install_compat()
    from concourse.bass_utils import run_bass_kernel_spmd

    x = np.asarray(x)
    edge_index = np.asarray(edge_index)
    W1 = np.asarray(W1, np.float32)
    b1v = np.asarray(b1, np.float32)
    W2 = np.asarray(W2, np.float32)
    b2v = np.asarray(b2, np.float32)
    n, in_dim = x.shape
    hid = W1.shape[1]
    out_dim = W2.shape[1]

    sch, nc = _get_compiled(n, edge_index.shape[1], edge_index, in_dim, hid, out_dim)
    in_maps = _make_in_maps(sch, x, W1, b1v, W2, b2v)
    import os

    trace = bool(os.environ.get("GCN_TRACE"))
    res = run_bass_kernel_spmd(
        nc, in_maps, core_ids=list(range(NCORES)), trace=trace
    )
    global LAST_EXEC_NS
    LAST_EXEC_NS = res.exec_time_ns
    return np.concatenate([res.results[c]["out"] for c in range(NCORES)], axis=0)


LAST_EXEC_NS = None


# revision 27
# speedup vs baseline: 1.2056x; 1.2056x over previous
"""2-layer GCN (GCNConv x2) on 8 Trainium2 NeuronCores.

Strategy (dst-sharded, edge-partitioned by destination; static-count gathers):
- Each core owns N/8 destination nodes and the edges pointing at them
  (plus the GCN self-loops, kept out of the edge stream).
- Layer-1 table: h~ = (dinv*x) @ W1 computed per-shard (x host-prescaled by
  dinv), ONE AllGather into hfull; gather banks = 4 int16-addressable row
  slices of hfull (rank-pair layout).
- Layer-2 table: h2~ = dinv * h2 written per-quarter (block-aligned
  quarters), 4 bank-wise AllGathers fired as quarters complete so they
  overlap layer-1's gather phase; bank q = concat over cores of quarter q.
- Per (super-block of SB_N dst blocks, bank): edges packed contiguously in
  block order into 128-slot chunks (straddling block boundaries); one
  dma_gather per segment on queue=bank (4 SWDGE queue contexts).
  Scatter-add via is_equal-indicator matmuls accumulating in PSUM.
- Gather counts are STATIC: every core gathers the max budget per segment,
  with idx 0 padding (gathers row 0 harmlessly; dstloc=-1 zeroes the
  indicator).  No per-gather reg_loads -> no WAR serialization through a
  shared register; the Q7 descgen queues pipeline freely.  num_idxs_reg
  comes from a small pool of registers preloaded once per distinct size.
- Layer-2 bank-3 gathers are DELAYED by L2_K3_DELAY super-blocks in issue
  order so the head of the in-order GpSimd stream never waits on the last
  AG2 (whose input is only ready when layer 1 finishes).
- Layer 1 accumulates transposed (aggT [feat, dst]) so bias+ReLU ride the
  activation engine and out1 feeds h2 = out1 @ W2 directly as lhsT.
"""
import sys
import types

import numpy as np
import ml_dtypes

P = 128
NCORES = 8
GMAX = 32  # max chunks (128 idxs each) per dma_gather
SB_N = 3  # dst blocks per super-block
NQUEUES = 4
XGRP = 25
GBUFS = 18
SBUFS = 10
AG2_LAG = 2  # super-blocks between a quarter finishing and its AG2 trigger
L2_K3_DELAY = 1  # super-blocks by which layer-2 bank-3 gathers trail
# (PSUM is bank-granular: 8 banks of 2KB; psagg needs (delay+1)*SB_N <= 6)
G_SB = 1  # super-blocks merged per dma_gather (1: per-idx Q7 cost dominates
# and small gathers pipeline better across the 4 queue contexts)

_CACHE = {}


# ---------------------------------------------------------------- compat ---
def _install_compat():
    """Patches for this axon/walrus stack (drain waits, per-inst wait caps,
    NTFF shim). Idempotent."""
    if _CACHE.get("compat"):
        return
    import concourse.tile as tile
    import concourse.mybir as mybir

    _ev = [0]

    def _split_inst_waits(ordered):
        for _bb, insts in ordered.items():
            out = []
            for inst in insts:
                si = getattr(inst, "sync_info", None)
                if si is not None and si.on_wait is not None and len(si.on_wait) > 1:
                    waits = list(si.on_wait)
                    excess, keep = waits[:-1], waits[-1:]
                    si.on_wait.clear()
                    for sw in keep:
                        si.on_wait.append(sw)
                    for i in range(0, len(excess), 2):
                        _ev[0] += 1
                        ev = mybir.InstEventSemaphore(
                            name=f"evsplit-{_ev[0]}", ins=[], outs=[]
                        )
                        ev.engine = inst.engine
                        ev.sync_info = mybir.SyncInfo(
                            on_wait=excess[i : i + 2], on_update=[]
                        )
                        out.append(ev)
                out.append(inst)
            insts[:] = out

    orig_lower = tile.TileContext._lower_ordered_insts

    def patched_lower(self, ordered):
        _split_inst_waits(ordered)
        return orig_lower(self, ordered)

    def patched_drain(self, tick_clock, wait_clock):
        sems_alloc = list(self.sems.allocated().values())
        carrier = self.nc.sync.wait_ge(sems_alloc[0], 0)
        wait_clock.add_sem_waits(
            carrier.ins, tile.ScopedClock({None: tick_clock.global_clock})
        )
        waits = list(carrier.ins.sync_info.on_wait)
        carrier.ins.sync_info.on_wait.clear()
        for sw in waits[:2]:
            carrier.ins.sync_info.on_wait.append(sw)
        for i in range(2, len(waits), 2):
            c = self.nc.sync.wait_ge(sems_alloc[0], 0)
            c.ins.sync_info.on_wait.clear()
            for sw in waits[i : i + 2]:
                c.ins.sync_info.on_wait.append(sw)
        self.nc.sync.drain(fusable=False)
        self.nc.all_engine_barrier()
        popped = self.nc._tile_sem_poison_stack.pop()
        assert popped is self._sem_poison
        self.nc.clear_and_free_semaphores(sems_alloc)
        self.nc.all_engine_barrier()

    tile.TileContext._lower_ordered_insts = patched_lower
    tile.TileContext._drain_and_barrier = patched_drain

    # NTFF profile hook shim (missing antenv.axon_hooks in this image)
    _hook = {}
    mod = types.ModuleType("antenv.axon_hooks")
    mod.set_axon_ntff_profile_hook = lambda h: _hook.update(hook=h)
    mod.get_axon_ntff_profile_hook = lambda: _hook.get("hook")
    sys.modules["antenv.axon_hooks"] = mod
    try:
        import antenv

        antenv.axon_hooks = mod
        from trn_agent_boot.trn_boot import _ntff_profile_via_ctypes

        mod.set_axon_ntff_profile_hook(
            _ntff_profile_via_ctypes("/opt/axon/libaxon_pjrt.so")
        )
    except Exception:
        pass
    _CACHE["compat"] = True


# ---------------------------------------------------------- preprocessing ---
class Schedule:
    pass


class LayerSched:
    pass


def _quarter_bounds(nblk):
    base, rem = nblk // 4, nblk % 4
    sizes = [base + (1 if i < rem else 0) for i in range(4)]
    starts = np.cumsum([0] + sizes)
    return [(int(starts[i]), int(starts[i + 1])) for i in range(4)]


def _make_layer_sched(
    n, nblk, n_sb, s_core, s_pair, s_bank, s_blk, s_bidx, s_dstloc
):
    """Build the per-(super-block-pair, bank) straddle-packed schedule for
    one bank mapping.  Inputs are edge arrays sorted by (core, pair, bank,
    blk).  Gather counts are static: all cores gather the full per-segment
    budget, with zero-index padding (row 0) beyond their real edges.
    Each gather's slots are split into per-super-block groups so indicator
    tiles and PE matmuls stay super-block granular."""
    e = s_core.shape[0]
    npair = (n_sb + G_SB - 1) // G_SB
    key = ((s_core * npair + s_pair) * 4 + s_bank) * nblk + s_blk
    cnt = np.bincount(key, minlength=NCORES * npair * 4 * nblk).reshape(
        NCORES, npair, 4, nblk
    )

    gathers = []
    slots = []
    chunk_gid = 0
    budget_tab = np.zeros((npair, 4), np.int64)
    for p in range(npair):
        blocks = list(
            range(p * G_SB * SB_N, min((p + 1) * G_SB * SB_N, nblk))
        )
        for k in range(4):
            percore = cnt[:, p, k, :][:, blocks]
            cum = np.cumsum(percore, axis=1)
            budget = max(int(np.ceil(cum[:, -1].max() / P)), 1)
            budget_tab[p, k] = budget
            lo = np.min(cum - percore, axis=0)
            hi = np.max(cum, axis=0)
            g0 = 0
            while g0 < budget:
                gn = min(GMAX, budget - g0)
                gi = len(gathers)
                gsl = []
                for j in range(g0, g0 + gn):
                    c_lo, c_hi = j * P, (j + 1) * P
                    for bi, b in enumerate(blocks):
                        if hi[bi] > c_lo and lo[bi] < c_hi:
                            gsl.append(
                                dict(
                                    g=gi,
                                    cl=j - g0,
                                    blk=b,
                                    sb=b // SB_N,
                                    bank=k,
                                    chunk_gid=chunk_gid + j,
                                )
                            )
                gathers.append(
                    dict(
                        gi=gi,
                        pair=p,
                        bank=k,
                        chunk0=chunk_gid + g0,
                        nch=gn,
                        nidx=gn * P,
                        _gsl=gsl,
                        c16=(chunk_gid + g0) * P // 16,
                    )
                )
                g0 += gn
            chunk_gid += budget
    totc = chunk_gid
    tot_slots = totc * P

    # assign global slot ids grouped by (gather, sb): dstloc column order is
    # host-chosen, so per-(gather, sb) indicator slices stay contiguous even
    # though chunk block-ranges interleave super-blocks across cores.
    first_slot_of_block = {}
    last_slot_of_block = {}
    for g in gathers:
        gsl = g.pop("_gsl")
        groups = {}
        for sb_val in sorted({sl["sb"] for sl in gsl}):
            grp_slots = [sl for sl in gsl if sl["sb"] == sb_val]
            grp_start = len(slots)
            for i, sl in enumerate(grp_slots):
                sl["slot_gid"] = len(slots)
                sl["sl_in_grp"] = i
                first_slot_of_block.setdefault((sb_val, sl["blk"]), len(slots))
                last_slot_of_block[(sb_val, sl["blk"])] = len(slots)
                slots.append(sl)
            groups[sb_val] = [grp_start, len(slots), grp_slots]
        g["groups"] = groups
    nslots = len(slots)

    # the self-loop identity matmul opens each block's PSUM group (start);
    # the last slot closes it (stop).  Slot-id order matches PE emission
    # order (pair, bank, sb-group, chunk), so the max id per block is the
    # last matmul emitted for it.
    for i, sl in enumerate(slots):
        sl["start"] = False
        sl["stop"] = last_slot_of_block[(sl["sb"], sl["blk"])] == i
    has_slots = set(first_slot_of_block.keys())

    seg_key = (s_core * npair + s_pair) * 4 + s_bank
    seg_ptr = np.searchsorted(seg_key, np.arange(NCORES * npair * 4 + 1))
    seg_chunk0 = {}
    cg = 0
    for p in range(npair):
        for k in range(4):
            seg_chunk0[(p, k)] = cg
            cg += int(budget_tab[p, k])

    # idx streams: zero padding beyond each core's real edges (gathers row 0;
    # dstloc stays -1 so the indicator kills the contribution).  Counts are
    # therefore identical across cores -> no dynamic trim registers.
    idx_flat = np.zeros((NCORES, tot_slots), np.int16)
    for c in range(NCORES):
        arr = idx_flat[c]
        for p in range(npair):
            for k in range(4):
                p0 = seg_ptr[(c * npair + p) * 4 + k]
                p1 = seg_ptr[(c * npair + p) * 4 + k + 1]
                base = seg_chunk0[(p, k)] * P
                arr[base : base + p1 - p0] = s_bidx[p0:p1].astype(np.int16)

    dstloc_s = np.full((NCORES, P, nslots), -1.0, np.float32)
    seg_id = (s_core * npair + s_pair) * 4 + s_bank
    pos_in_seg = np.arange(e) - seg_ptr[seg_id]
    seg_chunk0_arr = np.zeros(NCORES * npair * 4, np.int64)
    for p in range(npair):
        for k in range(4):
            for c in range(NCORES):
                seg_chunk0_arr[(c * npair + p) * 4 + k] = seg_chunk0[(p, k)]
    chunk_of_edge = seg_chunk0_arr[seg_id] + pos_in_seg // P
    part_of_edge = pos_in_seg % P
    slot_lut = np.full((totc, nblk), -1, np.int64)
    for i, sl in enumerate(slots):
        slot_lut[sl["chunk_gid"], sl["blk"]] = i
    slot_of_edge = slot_lut[chunk_of_edge, s_blk]
    assert (slot_of_edge >= 0).all()
    dstloc_s[s_core, part_of_edge, slot_of_edge] = s_dstloc

    idx_stream = np.ascontiguousarray(
        idx_flat.reshape(NCORES, tot_slots // 16, 16).transpose(0, 2, 1)
    )
    idx_stream = np.tile(idx_stream, (1, 8, 1))

    ls = LayerSched()
    ls.npair = npair
    ls.has_slots = has_slots
    ls.gathers = gathers
    ls.slots = slots
    ls.totc, ls.nslots, ls.tot_slots = totc, nslots, tot_slots
    # max per-(gather, sb) group size -> indicator tile width
    ls.grpmax = max(
        (grp[1] - grp[0] for g in gathers for grp in g["groups"].values()),
        default=1,
    )
    ls.gchmax = max(g["nch"] for g in gathers)
    ls.idx_stream = idx_stream
    ls.idx_flat = idx_flat
    ls.dstloc_s = dstloc_s.astype(ml_dtypes.bfloat16)
    return ls


def _preprocess(n, edge_index):
    src0 = np.asarray(edge_index[0], np.int64)
    dst0 = np.asarray(edge_index[1], np.int64)
    shard = n // NCORES
    nblk = (shard + P - 1) // P
    n_sb = (nblk + SB_N - 1) // SB_N
    qb = _quarter_bounds(nblk)
    qrow_start = [b0 * P for b0, b1 in qb]
    qrows = [min(b1 * P, shard) - b0 * P for b0, b1 in qb]
    bank2_n = [NCORES * r for r in qrows]
    assert all(b <= 32767 for b in bank2_n)
    bank1_rows = (n + 3) // 4  # rank-pair banks for layer 1 (slices of hfull)
    assert bank1_rows <= 32767

    deg = np.bincount(dst0, minlength=n).astype(np.float64) + 1.0
    dinv = (1.0 / np.sqrt(deg)).astype(np.float32)

    # self-loops are NOT in the edge stream: both layers fold them in with
    # one identity matmul per block from the per-core hloc/h2loc tensors.
    src = src0
    dst = dst0

    core_d = dst // shard
    dl = dst - core_d * shard
    blk = dl // P
    dstloc = (dl % P).astype(np.int64)
    sb = blk // SB_N

    # layer-1 bank mapping: contiguous row slices of hfull (rank-major)
    bank_a = src // bank1_rows
    bidx_a = src - bank_a * bank1_rows

    # layer-2 bank mapping: quarter-stacked
    core_s = src // shard
    off = src - core_s * shard
    sblk = off // P
    qb_arr = np.zeros(nblk, np.int64)
    for q, (b0, b1) in enumerate(qb):
        qb_arr[b0:b1] = q
    bank_b = qb_arr[sblk]
    bidx_b = core_s * np.array(qrows)[bank_b] + (off - np.array(qrow_start)[bank_b])

    pair = sb // G_SB
    scheds = []
    for bank, bidx in ((bank_a, bidx_a), (bank_b, bidx_b)):
        order = np.lexsort((blk, bank, pair, core_d))
        scheds.append(
            _make_layer_sched(
                n,
                nblk,
                n_sb,
                core_d[order],
                pair[order],
                bank[order],
                blk[order],
                bidx[order],
                dstloc[order],
            )
        )

    sch = Schedule()
    sch.n, sch.shard, sch.nblk, sch.n_sb = n, shard, nblk, n_sb
    sch.e = src.shape[0]
    sch.qb, sch.qrow_start, sch.qrows = qb, qrow_start, qrows
    sch.bank1_rows, sch.bank2_n = bank1_rows, bank2_n
    sch.dinv = dinv
    sch.L1, sch.L2 = scheds
    sch.grpmax = max(sch.L1.grpmax, sch.L2.grpmax)
    sch.gchmax = max(sch.L1.gchmax, sch.L2.gchmax)
    sch.nidx_vals = sorted(
        {g["nidx"] for g in sch.L1.gathers} | {g["nidx"] for g in sch.L2.gathers}
    )
    return sch


# ----------------------------------------------------------------- build ---
def _build(sch, in_dim, hid, out_dim):
    import concourse.mybir as mybir
    import concourse.tile as tile
    from concourse import bacc

    bf16 = mybir.dt.bfloat16
    f32 = mybir.dt.float32
    shard, nblk, n_sb = sch.shard, sch.nblk, sch.n_sb
    grpmax = sch.grpmax
    gchmax = sch.gchmax
    qb = sch.qb
    n = sch.n

    nc = bacc.Bacc(num_swdge_queues=NQUEUES)

    xT = nc.declare_dram_parameter("xT", [in_dim, shard], bf16, isOutput=False)
    idxs1 = nc.declare_dram_parameter(
        "idxs1", [P, sch.L1.tot_slots // 16], mybir.dt.int16, isOutput=False
    )
    idxs2 = nc.declare_dram_parameter(
        "idxs2", [P, sch.L2.tot_slots // 16], mybir.dt.int16, isOutput=False
    )
    dstloc1 = nc.declare_dram_parameter(
        "dstloc1", [P, sch.L1.nslots], bf16, isOutput=False
    )
    dstloc2 = nc.declare_dram_parameter(
        "dstloc2", [P, sch.L2.nslots], bf16, isOutput=False
    )
    iotar_in = nc.declare_dram_parameter("iotar", [P, grpmax * P], bf16, isOutput=False)
    dinvbc = nc.declare_dram_parameter("dinvbc", [P, nblk * P], f32, isOutput=False)
    dinvb = nc.declare_dram_parameter("dinvb", [P, nblk], f32, isOutput=False)
    w1 = nc.declare_dram_parameter("W1", [in_dim, hid], bf16, isOutput=False)
    b1 = nc.declare_dram_parameter("b1", [hid, 1], f32, isOutput=False)
    w2 = nc.declare_dram_parameter("W2", [hid, out_dim], bf16, isOutput=False)
    b2bc = nc.declare_dram_parameter("b2bc", [P, out_dim], f32, isOutput=False)
    ident_in = nc.declare_dram_parameter("ident", [P, P], bf16, isOutput=False)
    out_ext = nc.declare_dram_parameter("out", [shard, out_dim], f32, isOutput=True)

    dummy_tbl = nc.dram_tensor("dummy_tbl", [1, P], bf16)
    hloc = nc.dram_tensor("hloc", [shard, P], bf16)
    hfull = nc.dram_tensor("hfull", [n, P], bf16, addr_space="Shared")
    h2loc_q = [
        nc.dram_tensor(f"h2loc{q}", [sch.qrows[q], P], bf16) for q in range(4)
    ]
    h2bank = [
        nc.dram_tensor(f"h2bank{q}", [sch.bank2_n[q], P], bf16, addr_space="Shared")
        for q in range(4)
    ]

    kin = in_dim // P

    def quarter_of(b):
        for q, (b0, b1) in enumerate(qb):
            if b0 <= b < b1:
                return q
        raise AssertionError

    def layer_maps(ls):
        seg_gathers = {}
        for g in ls.gathers:
            seg_gathers.setdefault((g["pair"], g["bank"]), []).append(g)
        return seg_gathers

    seg1 = layer_maps(sch.L1)
    seg2 = layer_maps(sch.L2)

    ag2_at_sb = {}
    for q in range(4):
        sq_end = (qb[q][1] - 1) // SB_N
        key = min(sq_end + AG2_LAG, n_sb - 1) if q < 3 else n_sb - 1
        ag2_at_sb.setdefault(key, []).append(q)

    with tile.TileContext(nc) as tc:
        with (
            tc.tile_pool(name="const", bufs=1) as cpool,
            tc.tile_pool(name="xload", bufs=2) as xpool,
            tc.tile_pool(name="hb", bufs=3) as hbpool,
            tc.tile_pool(name="idx", bufs=16) as ipool,
            tc.tile_pool(name="gath", bufs=GBUFS) as gpool,
            tc.tile_pool(name="sind", bufs=SBUFS) as spool,
            tc.tile_pool(name="dvp", bufs=3) as dvpool,
            tc.tile_pool(name="rl", bufs=8) as rpool,
            tc.tile_pool(name="blk", bufs=3) as bpool,
            tc.tile_pool(name="ob", bufs=2) as opool,
            tc.tile_pool(name="psh", bufs=1, space="PSUM") as psh,
            tc.tile_pool(
                name="psagg", bufs=(L2_K3_DELAY + 1) * SB_N, space="PSUM"
            ) as psagg,
            tc.tile_pool(name="psh2", bufs=1, space="PSUM") as psh2,
        ):
            import contextlib

            regstack = contextlib.ExitStack()

            # ---- constants into SBUF
            w1_t = [
                cpool.tile([P, hid], bf16, tag=f"w1_{k}", name=f"w1t{k}")
                for k in range(kin)
            ]
            for k in range(kin):
                nc.sync.dma_start(out=w1_t[k][:], in_=w1[k * P : (k + 1) * P, :])
            w2_sb = cpool.tile([hid, out_dim], bf16, tag="w2")
            nc.sync.dma_start(out=w2_sb[:], in_=w2[:])
            b1_sb = cpool.tile([hid, 1], f32, tag="b1")
            nc.sync.dma_start(out=b1_sb[:], in_=b1[:])
            b2_sb = cpool.tile([P, out_dim], f32, tag="b2")
            nc.sync.dma_start(out=b2_sb[:], in_=b2bc[:])
            dinvb_sb = cpool.tile([P, nblk], f32, tag="dinvb")
            nc.sync.dma_start(out=dinvb_sb[:], in_=dinvb[:])
            dstloc1_sb = cpool.tile([P, sch.L1.nslots], bf16, tag="dstloc1")
            nc.sync.dma_start(out=dstloc1_sb[:], in_=dstloc1[:])
            dstloc2_sb = cpool.tile([P, sch.L2.nslots], bf16, tag="dstloc2")
            nc.sync.dma_start(out=dstloc2_sb[:], in_=dstloc2[:])
            iotar_sb = cpool.tile([P, grpmax * P], bf16, tag="iotar")
            nc.sync.dma_start(out=iotar_sb[:], in_=iotar_in[:])
            ident_sb = cpool.tile([P, P], bf16, tag="ident")
            nc.sync.dma_start(out=ident_sb[:], in_=ident_in[:])

            # ---- h~ = (dinv*x) @ W1, shard-local (x pre-scaled by dinv);
            # 4 blocks batched per PSUM bank, double-buffered
            for g0 in range(0, nblk, XGRP):
                g1 = min(g0 + XGRP, nblk)
                c0, c1 = g0 * P, min(g1 * P, shard)
                xt = [
                    xpool.tile([P, XGRP * P], bf16, tag=f"xt{k}", name=f"xt{k}")
                    for k in range(kin)
                ]
                for k in range(kin):
                    nc.sync.dma_start(
                        out=xt[k][:, : c1 - c0], in_=xT[k * P : (k + 1) * P, c0:c1]
                    )
                for s0 in range(g0, g1, 4):
                    s1 = min(s0 + 4, g1)
                    hp = psh.tile([P, 4 * hid], f32, tag="hps")
                    mtot = min(s1 * P, shard) - s0 * P
                    for b in range(s0, s1):
                        m = min(P, shard - b * P)
                        sub = b - s0
                        for k in range(kin):
                            nc.tensor.matmul(
                                out=hp[:m, sub * hid : (sub + 1) * hid],
                                lhsT=xt[k][:, b * P - c0 : b * P - c0 + m],
                                rhs=w1_t[k][:],
                                start=(k == 0),
                                stop=(k == kin - 1),
                            )
                    nsub = s1 - s0
                    hsb = hbpool.tile([P, 4, hid], bf16, tag="hsb")
                    nc.scalar.activation(
                        out=hsb[:, :nsub, :],
                        in_=hp[:, : nsub * hid].rearrange("p (g f) -> p g f", g=nsub),
                        func=mybir.ActivationFunctionType.Copy,
                    )
                    nfull = mtot // P
                    if nfull:
                        nc.sync.dma_start(
                            out=hloc[s0 * P : s0 * P + nfull * P, :].rearrange(
                                "(g p) f -> p g f", p=P
                            ),
                            in_=hsb[:, :nfull, :],
                        )
                    if mtot > nfull * P:
                        mp = mtot - nfull * P
                        nc.sync.dma_start(
                            out=hloc[s0 * P + nfull * P : s0 * P + mtot, :],
                            in_=hsb[:mp, nfull, :],
                        )

            nc.gpsimd.collective_compute(
                "AllGather",
                mybir.AluOpType.bypass,
                ins=[hloc[:]],
                outs=[hfull[:]],
                replica_groups=[list(range(NCORES))],
            )

            def bank_table(layer, k):
                if layer == 1:
                    r0 = k * sch.bank1_rows
                    r1 = min(r0 + sch.bank1_rows, n)
                    return hfull[r0:r1, :]
                return h2bank[k][0 : sch.bank2_n[k], :]

            def issue_gather(layer, g, it):
                nidx, nch = g["nidx"], g["nch"]
                gt = gpool.tile([P, gchmax, P], bf16, tag="gt")
                nc.gpsimd.dma_gather(
                    out_ap=gt[:, :nch, :],
                    in_ap=bank_table(layer, g["bank"]),
                    idxs_ap=it[:, : nidx // 16],
                    num_idxs=nidx,
                    num_idxs_reg=nidx,
                    elem_size=P,
                    single_packet=False,
                    queue_num=g["bank"],
                )
                return gt

            def load_it(layer, g):
                idxs = idxs1 if layer == 1 else idxs2
                it = ipool.tile([P, gchmax * 8], mybir.dt.int16, tag="it")
                nc.sync.dma_start(
                    out=it[:, : g["nidx"] // 16],
                    in_=idxs[:, g["c16"] : g["c16"] + g["nidx"] // 16],
                )
                return it

            def build_ind(grp_start, grp_end, dstloc_sb):
                nsl = grp_end - grp_start
                sbig = spool.tile([P, grpmax, P], bf16, tag="sind")
                nc.vector.tensor_tensor(
                    out=sbig[:, :nsl, :],
                    in0=iotar_sb[:, : nsl * P].rearrange("p (k f) -> p k f", k=nsl),
                    in1=dstloc_sb[:, grp_start:grp_end].to_broadcast([P, nsl, P]),
                    op=mybir.AluOpType.is_equal,
                )
                return sbig

            def run_layer(layer):
                """Gathers are merged per (super-block pair, bank); slot
                matmuls and epilogues stay per super-block.  Layer 2 delays
                bank-3 gathers by one step so the in-order GpSimd stream
                never parks on the last AG2."""
                seg_gathers = seg1 if layer == 1 else seg2
                ls = sch.L1 if layer == 1 else sch.L2
                dstloc_sb = dstloc1_sb if layer == 1 else dstloc2_sb
                delay = 0 if layer == 1 else L2_K3_DELAY
                w = P if layer == 1 else out_dim

                def issued_at(t):
                    """gathers whose dma_gather is issued at step t."""
                    out = []
                    if t < n_sb and t % G_SB == 0:
                        p = t // G_SB
                        ks = range(4) if layer == 1 else range(3)
                        for k in ks:
                            out.extend(seg_gathers.get((p, k), []))
                    if layer == 2 and 0 <= t - 1 < n_sb and (t - 1) % G_SB == 0:
                        p = (t - 1) // G_SB
                        out.extend(seg_gathers.get((p, 3), []))
                    return out

                def slot_groups_at(t):
                    """(gather, sb) slot groups whose matmuls run at step t."""
                    out = []
                    if t < n_sb:
                        p = t // G_SB
                        ks = range(4) if layer == 1 else range(3)
                        for k in ks:
                            for g in seg_gathers.get((p, k), []):
                                if t in g["groups"]:
                                    out.append((g, t))
                    if layer == 2 and 0 <= t - 1 < n_sb:
                        p = (t - 1) // G_SB
                        for g in seg_gathers.get((p, 3), []):
                            if t - 1 in g["groups"]:
                                out.append((g, t - 1))
                    return out

                def prefetch_it(t):
                    return {g["gi"]: load_it(layer, g) for g in issued_at(t)}

                def prefetch_ind(t):
                    tiles = {}
                    for g, s in slot_groups_at(t):
                        grp = g["groups"][s]
                        tiles[(g["gi"], s)] = build_ind(grp[0], grp[1], dstloc_sb)
                    return tiles

                def prefetch_rl(s):
                    tiles = {}
                    if not (0 <= s < n_sb):
                        return tiles
                    blocks = list(range(s * SB_N, min((s + 1) * SB_N, nblk)))
                    # batch contiguous full blocks within one source tensor
                    runs = []
                    for b in blocks:
                        q = quarter_of(b) if layer == 2 else 0
                        if runs and runs[-1][2] == q and runs[-1][1] == b:
                            runs[-1][1] = b + 1
                        else:
                            runs.append([b, b + 1, q])
                    for b0, b1, q in runs:
                        nb = b1 - b0
                        m_end = min(b1 * P, shard) - b0 * P
                        rl = rpool.tile([P, SB_N, P], bf16, tag="rl")
                        src = hloc if layer == 1 else h2loc_q[q]
                        r0 = b0 * P - (0 if layer == 1 else sch.qrow_start[q])
                        nfull = m_end // P
                        if nfull:
                            nc.sync.dma_start(
                                out=rl[:, :nfull, :],
                                in_=src[r0 : r0 + nfull * P, :].rearrange(
                                    "(g p) f -> p g f", p=P
                                ),
                            )
                        if m_end > nfull * P:
                            mp = m_end - nfull * P
                            nc.sync.dma_start(
                                out=rl[:mp, nfull, :],
                                in_=src[r0 + nfull * P : r0 + m_end, :],
                            )
                        for b in range(b0, b1):
                            tiles[b] = (rl, b - b0)
                    return tiles

                def prefetch_dv(s):
                    if layer != 1 or not (0 <= s < n_sb):
                        return None
                    b0 = s * SB_N
                    b1 = min(b0 + SB_N, nblk)
                    dv = dvpool.tile([P, SB_N * P], f32, tag="dv")
                    nc.sync.dma_start(
                        out=dv[:, : (b1 - b0) * P], in_=dinvbc[:, b0 * P : b1 * P]
                    )
                    return dv

                it_tiles = prefetch_it(0)
                ind_tiles = prefetch_ind(0)
                rl_tiles = prefetch_rl(0)
                dv_cur = prefetch_dv(0)
                rl_hold = {}
                dv_hold = {}
                agg_hold = {}
                gt_tiles = {}
                nsteps = n_sb + delay
                for t in range(nsteps):
                    s_new = t if t < n_sb else None
                    s_old = t - delay if 0 <= t - delay < n_sb else None
                    next_it = prefetch_it(t + 1)
                    next_ind = prefetch_ind(t + 1)
                    next_rl = prefetch_rl(t + 1)
                    next_dv = prefetch_dv(t + 1)

                    if s_new is not None:
                        blocks = list(
                            range(s_new * SB_N, min((s_new + 1) * SB_N, nblk))
                        )
                        agg_t = {
                            b: psagg.tile(
                                [P, w], f32, tag="agg", name=f"agg{layer}_{s_new}_{b}"
                            )
                            for b in blocks
                        }
                        agg_hold[s_new] = agg_t
                        rl_hold[s_new] = rl_tiles
                        dv_hold[s_new] = dv_cur
                        # self-loop contribution opens each block's PSUM group
                        for b in blocks:
                            m = min(P, shard - b * P)
                            rl, sub = rl_tiles[b]
                            solo = (s_new, b) not in ls.has_slots
                            if layer == 1:
                                nc.tensor.matmul(
                                    out=agg_t[b][:, :],
                                    lhsT=rl[:m, sub, :],
                                    rhs=ident_sb[:m, :],
                                    start=True,
                                    stop=solo,
                                )
                            else:
                                nc.tensor.matmul(
                                    out=agg_t[b][:, :],
                                    lhsT=ident_sb[:m, :],
                                    rhs=rl[:m, sub, :out_dim],
                                    start=True,
                                    stop=solo,
                                )

                    for g in issued_at(t):
                        gt_tiles[g["gi"]] = issue_gather(
                            layer, g, it_tiles[g["gi"]]
                        )

                    for g, s in slot_groups_at(t):
                        gt = gt_tiles[g["gi"]]
                        sbig = ind_tiles[(g["gi"], s)]
                        agg_t = agg_hold[s]
                        for sl in g["groups"][s][2]:
                            if layer == 1:
                                nc.tensor.matmul(
                                    out=agg_t[sl["blk"]][:, :],
                                    lhsT=gt[:, sl["cl"], :],
                                    rhs=sbig[:, sl["sl_in_grp"], :],
                                    start=sl["start"],
                                    stop=sl["stop"],
                                )
                            else:
                                nc.tensor.matmul(
                                    out=agg_t[sl["blk"]][:, :],
                                    lhsT=sbig[:, sl["sl_in_grp"], :],
                                    rhs=gt[:, sl["cl"], :out_dim],
                                    start=sl["start"],
                                    stop=sl["stop"],
                                )

                    # ---- block epilogues for the super-block closing now
                    if s_old is not None:
                        blocks = list(
                            range(s_old * SB_N, min((s_old + 1) * SB_N, nblk))
                        )
                        agg_t = agg_hold.pop(s_old)
                        dv_sb = dv_hold.pop(s_old)
                        rl_hold.pop(s_old)
                        ob = (
                            opool.tile([P, SB_N, out_dim], f32, tag="ob", name="ob")
                            if layer == 2
                            else None
                        )
                        for b in blocks:
                            m = min(P, shard - b * P)
                            off = (b - s_old * SB_N) * P
                            if layer == 1:
                                t1 = bpool.tile([P, P], bf16, tag="t1")
                                nc.vector.tensor_tensor(
                                    out=t1[:],
                                    in0=agg_t[b][:, :],
                                    in1=dv_sb[:, off : off + P],
                                    op=mybir.AluOpType.mult,
                                )
                                o1 = bpool.tile([P, P], bf16, tag="o1")
                                nc.scalar.activation(
                                    out=o1[:],
                                    in_=t1[:],
                                    func=mybir.ActivationFunctionType.Relu,
                                    bias=b1_sb[:, :1],
                                )
                                h2p = psh2.tile([P, out_dim], f32, tag="h2p")
                                nc.tensor.matmul(
                                    out=h2p[:],
                                    lhsT=o1[:],
                                    rhs=w2_sb[:],
                                    start=True,
                                    stop=True,
                                )
                                h2s = bpool.tile([P, P], bf16, tag="h2s")
                                nc.scalar.activation(
                                    out=h2s[:m, :out_dim],
                                    in_=h2p[:m, :],
                                    func=mybir.ActivationFunctionType.Copy,
                                    scale=dinvb_sb[:m, b : b + 1],
                                )
                                q = quarter_of(b)
                                r0 = b * P - sch.qrow_start[q]
                                nc.sync.dma_start(
                                    out=h2loc_q[q][r0 : r0 + m, 0:out_dim],
                                    in_=h2s[:m, :out_dim],
                                )
                            else:
                                sub = b - blocks[0]
                                t2 = bpool.tile([P, out_dim], f32, tag="t2")
                                nc.scalar.activation(
                                    out=t2[:m, :],
                                    in_=agg_t[b][:m, :],
                                    func=mybir.ActivationFunctionType.Copy,
                                    scale=dinvb_sb[:m, b : b + 1],
                                )
                                nc.vector.tensor_tensor(
                                    out=ob[:m, sub, :],
                                    in0=t2[:m, :],
                                    in1=b2_sb[:m, :],
                                    op=mybir.AluOpType.add,
                                )
                        if layer == 2:
                            b0 = blocks[0]
                            m_end = min(blocks[-1] * P + P, shard) - b0 * P
                            nfull = m_end // P
                            if nfull:
                                nc.sync.dma_start(
                                    out=out_ext[
                                        b0 * P : b0 * P + nfull * P, :
                                    ].rearrange("(g p) f -> p g f", p=P),
                                    in_=ob[:, :nfull, :],
                                )
                            if m_end > nfull * P:
                                mp = m_end - nfull * P
                                nc.sync.dma_start(
                                    out=out_ext[b0 * P + nfull * P : b0 * P + m_end, :],
                                    in_=ob[:mp, nfull, :],
                                )

                    if layer == 1 and s_old is not None:
                        for q in ag2_at_sb.get(s_old, []):
                            nc.gpsimd.collective_compute(
                                "AllGather",
                                mybir.AluOpType.bypass,
                                ins=[h2loc_q[q][:]],
                                outs=[h2bank[q][:]],
                                replica_groups=[list(range(NCORES))],
                            )

                    it_tiles = next_it
                    ind_tiles = next_ind
                    rl_tiles = next_rl
                    dv_cur = next_dv

            run_layer(1)
            run_layer(2)
            regstack.close()

    nc.compile()
    return nc


# ---------------------------------------------------------------- kernel ---
def _make_in_maps(sch, x, W1, b1v, W2, b2v):
    hid = W1.shape[1]
    out_dim = W2.shape[1]
    shard, nblk = sch.shard, sch.nblk
    bf = ml_dtypes.bfloat16
    in_maps = []
    w1b = W1.astype(bf)
    w2b = W2.astype(bf)
    b1c = b1v.reshape(hid, 1).astype(np.float32).copy()
    b2c = np.broadcast_to(b2v.astype(np.float32), (P, out_dim)).copy()
    iotar = np.tile(np.arange(P, dtype=np.float32), (P, sch.grpmax)).astype(bf)
    ident = np.eye(P, dtype=np.float32).astype(bf)
    xs_all = (x * sch.dinv[:, None]).astype(bf)
    for c in range(NCORES):
        xs = np.ascontiguousarray(xs_all[c * shard : (c + 1) * shard].T)
        dv = sch.dinv[c * shard : (c + 1) * shard]
        full = np.zeros(nblk * P, np.float32)
        full[:shard] = dv
        dvb = np.ascontiguousarray(full.reshape(nblk, P).T)
        dbc = np.broadcast_to(full, (P, nblk * P)).copy()
        in_maps.append(
            {
                "xT": xs,
                "idxs1": sch.L1.idx_stream[c],
                "idxs2": sch.L2.idx_stream[c],
                "dstloc1": sch.L1.dstloc_s[c],
                "dstloc2": sch.L2.dstloc_s[c],
                "dinvb": dvb,
                "W1": w1b,
                "b1": b1c,
                "W2": w2b,
                "b2bc": b2c,
                "iotar": iotar,
                "ident": ident,
                "dinvbc": dbc,
            }
        )
    return in_maps


def _get_compiled(n, e, edge_index, in_dim, hid, out_dim):
    key = ("nc", n, e)
    if key not in _CACHE:
        sch = _preprocess(n, edge_index)
        _CACHE[("sched", n, e)] = sch
        _CACHE[key] = _build(sch, in_dim, hid, out_dim)
    return _CACHE[("sched", n, e)], _CACHE[key]


def kernel(x, edge_index, W1, b1, W2, b2):
    _install_compat()
    from concourse.bass_utils import run_bass_kernel_spmd

    x = np.asarray(x)
    edge_index = np.asarray(edge_index)
    W1 = np.asarray(W1, np.float32)
    b1v = np.asarray(b1, np.float32)
    W2 = np.asarray(W2, np.float32)
    b2v = np.asarray(b2, np.float32)
    n, in_dim = x.shape
    hid = W1.shape[1]
    out_dim = W2.shape[1]

    sch, nc = _get_compiled(n, edge_index.shape[1], edge_index, in_dim, hid, out_dim)
    in_maps = _make_in_maps(sch, x, W1, b1v, W2, b2v)
    import os

    trace = bool(os.environ.get("GCN_TRACE"))
    res = run_bass_kernel_spmd(
        nc, in_maps, core_ids=list(range(NCORES)), trace=trace
    )
    global LAST_EXEC_NS
    LAST_EXEC_NS = res.exec_time_ns
    return np.concatenate([res.results[c]["out"] for c in range(NCORES)], axis=0)


LAST_EXEC_NS = None


# revision 42
# speedup vs baseline: 1.2206x; 1.0125x over previous
"""2-layer GCN (GCNConv x2) on 8 Trainium2 NeuronCores.

Strategy (dst-sharded, edge-partitioned by destination; static-count gathers):
- Each core owns N/8 destination nodes and the edges pointing at them
  (plus the GCN self-loops, kept out of the edge stream).
- Layer-1 table: h~ = (dinv*x) @ W1 computed per-shard (x host-prescaled by
  dinv), ONE AllGather into hfull; gather banks = 4 int16-addressable row
  slices of hfull (rank-pair layout).
- Layer-2 table: h2~ = dinv * h2 written per-quarter (block-aligned
  quarters), 4 bank-wise AllGathers fired as quarters complete so they
  overlap layer-1's gather phase; bank q = concat over cores of quarter q.
- Per (super-block of SB_N dst blocks, bank): edges packed contiguously in
  block order into 128-slot chunks (straddling block boundaries); one
  dma_gather per segment on queue=bank (4 SWDGE queue contexts).
  Scatter-add via is_equal-indicator matmuls accumulating in PSUM.
- Gather counts are STATIC: every core gathers the max budget per segment,
  with idx 0 padding (gathers row 0 harmlessly; dstloc=-1 zeroes the
  indicator).  No per-gather reg_loads -> no WAR serialization through a
  shared register; the Q7 descgen queues pipeline freely.  num_idxs_reg
  comes from a small pool of registers preloaded once per distinct size.
- Layer-2 bank-3 gathers are DELAYED by L2_K3_DELAY super-blocks in issue
  order so the head of the in-order GpSimd stream never waits on the last
  AG2 (whose input is only ready when layer 1 finishes).
- Layer 1 accumulates transposed (aggT [feat, dst]) so bias+ReLU ride the
  activation engine and out1 feeds h2 = out1 @ W2 directly as lhsT.
"""
import sys
import types

import numpy as np
import ml_dtypes

P = 128
NCORES = 8
GMAX = 32  # max chunks (128 idxs each) per dma_gather
SB_N = 3  # dst blocks per super-block
NQUEUES = 4
XGRP = 25
GBUFS = 18
SBUFS = 10
PREP_SB = 0  # prepare_only pre-generation wave disabled (caused a device
# hang; the PE-side manual sem sync with the SWDGE ring needs more work)
AG2_LAG = 2  # super-blocks between a quarter finishing and its AG2 trigger
L2_K3_DELAY = 1  # super-blocks by which layer-2 bank-3 gathers trail
# (PSUM is bank-granular: 8 banks of 2KB; psagg needs (delay+1)*SB_N <= 6)
G_SB = 1  # super-blocks merged per dma_gather (1: per-idx Q7 cost dominates
# and small gathers pipeline better across the 4 queue contexts)

_CACHE = {}


# ---------------------------------------------------------------- compat ---
def _install_compat():
    """Patches for this axon/walrus stack (drain waits, per-inst wait caps,
    NTFF shim). Idempotent."""
    if _CACHE.get("compat"):
        return
    import concourse.tile as tile
    import concourse.mybir as mybir

    _ev = [0]

    def _split_inst_waits(ordered):
        for _bb, insts in ordered.items():
            out = []
            for inst in insts:
                si = getattr(inst, "sync_info", None)
                if si is not None and si.on_wait is not None and len(si.on_wait) > 1:
                    waits = list(si.on_wait)
                    excess, keep = waits[:-1], waits[-1:]
                    si.on_wait.clear()
                    for sw in keep:
                        si.on_wait.append(sw)
                    for i in range(0, len(excess), 2):
                        _ev[0] += 1
                        ev = mybir.InstEventSemaphore(
                            name=f"evsplit-{_ev[0]}", ins=[], outs=[]
                        )
                        ev.engine = inst.engine
                        ev.sync_info = mybir.SyncInfo(
                            on_wait=excess[i : i + 2], on_update=[]
                        )
                        out.append(ev)
                out.append(inst)
            insts[:] = out

    orig_lower = tile.TileContext._lower_ordered_insts

    def patched_lower(self, ordered):
        _split_inst_waits(ordered)
        return orig_lower(self, ordered)

    def patched_drain(self, tick_clock, wait_clock):
        sems_alloc = list(self.sems.allocated().values())
        carrier = self.nc.sync.wait_ge(sems_alloc[0], 0)
        wait_clock.add_sem_waits(
            carrier.ins, tile.ScopedClock({None: tick_clock.global_clock})
        )
        waits = list(carrier.ins.sync_info.on_wait)
        carrier.ins.sync_info.on_wait.clear()
        for sw in waits[:2]:
            carrier.ins.sync_info.on_wait.append(sw)
        for i in range(2, len(waits), 2):
            c = self.nc.sync.wait_ge(sems_alloc[0], 0)
            c.ins.sync_info.on_wait.clear()
            for sw in waits[i : i + 2]:
                c.ins.sync_info.on_wait.append(sw)
        self.nc.sync.drain(fusable=False)
        self.nc.all_engine_barrier()
        popped = self.nc._tile_sem_poison_stack.pop()
        assert popped is self._sem_poison
        self.nc.clear_and_free_semaphores(sems_alloc)
        self.nc.all_engine_barrier()

    tile.TileContext._lower_ordered_insts = patched_lower
    tile.TileContext._drain_and_barrier = patched_drain

    # NTFF profile hook shim (missing antenv.axon_hooks in this image)
    _hook = {}
    mod = types.ModuleType("antenv.axon_hooks")
    mod.set_axon_ntff_profile_hook = lambda h: _hook.update(hook=h)
    mod.get_axon_ntff_profile_hook = lambda: _hook.get("hook")
    sys.modules["antenv.axon_hooks"] = mod
    try:
        import antenv

        antenv.axon_hooks = mod
        from trn_agent_boot.trn_boot import _ntff_profile_via_ctypes

        mod.set_axon_ntff_profile_hook(
            _ntff_profile_via_ctypes("/opt/axon/libaxon_pjrt.so")
        )
    except Exception:
        pass
    _CACHE["compat"] = True


# ---------------------------------------------------------- preprocessing ---
class Schedule:
    pass


class LayerSched:
    pass


def _quarter_bounds(nblk):
    base, rem = nblk // 4, nblk % 4
    sizes = [base + (1 if i < rem else 0) for i in range(4)]
    starts = np.cumsum([0] + sizes)
    return [(int(starts[i]), int(starts[i + 1])) for i in range(4)]


def _make_layer_sched(
    n, nblk, n_sb, s_core, s_pair, s_bank, s_blk, s_bidx, s_dstloc
):
    """Build the per-(super-block-pair, bank) straddle-packed schedule for
    one bank mapping.  Inputs are edge arrays sorted by (core, pair, bank,
    blk).  Gather counts are static: all cores gather the full per-segment
    budget, with zero-index padding (row 0) beyond their real edges.
    Each gather's slots are split into per-super-block groups so indicator
    tiles and PE matmuls stay super-block granular."""
    e = s_core.shape[0]
    npair = (n_sb + G_SB - 1) // G_SB
    key = ((s_core * npair + s_pair) * 4 + s_bank) * nblk + s_blk
    cnt = np.bincount(key, minlength=NCORES * npair * 4 * nblk).reshape(
        NCORES, npair, 4, nblk
    )

    gathers = []
    slots = []
    chunk_gid = 0
    budget_tab = np.zeros((npair, 4), np.int64)
    for p in range(npair):
        blocks = list(
            range(p * G_SB * SB_N, min((p + 1) * G_SB * SB_N, nblk))
        )
        for k in range(4):
            percore = cnt[:, p, k, :][:, blocks]
            cum = np.cumsum(percore, axis=1)
            budget = max(int(np.ceil(cum[:, -1].max() / P)), 1)
            budget_tab[p, k] = budget
            lo = np.min(cum - percore, axis=0)
            hi = np.max(cum, axis=0)
            g0 = 0
            while g0 < budget:
                gn = min(GMAX, budget - g0)
                gi = len(gathers)
                gsl = []
                for j in range(g0, g0 + gn):
                    c_lo, c_hi = j * P, (j + 1) * P
                    for bi, b in enumerate(blocks):
                        if hi[bi] > c_lo and lo[bi] < c_hi:
                            gsl.append(
                                dict(
                                    g=gi,
                                    cl=j - g0,
                                    blk=b,
                                    sb=b // SB_N,
                                    bank=k,
                                    chunk_gid=chunk_gid + j,
                                )
                            )
                gathers.append(
                    dict(
                        gi=gi,
                        pair=p,
                        bank=k,
                        chunk0=chunk_gid + g0,
                        nch=gn,
                        nidx=gn * P,
                        _gsl=gsl,
                        c16=(chunk_gid + g0) * P // 16,
                    )
                )
                g0 += gn
            chunk_gid += budget
    totc = chunk_gid
    tot_slots = totc * P

    # assign global slot ids grouped by (gather, sb): dstloc column order is
    # host-chosen, so per-(gather, sb) indicator slices stay contiguous even
    # though chunk block-ranges interleave super-blocks across cores.
    first_slot_of_block = {}
    last_slot_of_block = {}
    for g in gathers:
        gsl = g.pop("_gsl")
        groups = {}
        for sb_val in sorted({sl["sb"] for sl in gsl}):
            grp_slots = [sl for sl in gsl if sl["sb"] == sb_val]
            grp_start = len(slots)
            for i, sl in enumerate(grp_slots):
                sl["slot_gid"] = len(slots)
                sl["sl_in_grp"] = i
                first_slot_of_block.setdefault((sb_val, sl["blk"]), len(slots))
                last_slot_of_block[(sb_val, sl["blk"])] = len(slots)
                slots.append(sl)
            groups[sb_val] = [grp_start, len(slots), grp_slots]
        g["groups"] = groups
    nslots = len(slots)

    # the self-loop identity matmul opens each block's PSUM group (start);
    # the last slot closes it (stop).  Slot-id order matches PE emission
    # order (pair, bank, sb-group, chunk), so the max id per block is the
    # last matmul emitted for it.
    for i, sl in enumerate(slots):
        sl["start"] = False
        sl["stop"] = last_slot_of_block[(sl["sb"], sl["blk"])] == i
    has_slots = set(first_slot_of_block.keys())

    seg_key = (s_core * npair + s_pair) * 4 + s_bank
    seg_ptr = np.searchsorted(seg_key, np.arange(NCORES * npair * 4 + 1))
    seg_chunk0 = {}
    cg = 0
    for p in range(npair):
        for k in range(4):
            seg_chunk0[(p, k)] = cg
            cg += int(budget_tab[p, k])

    # idx streams: zero padding beyond each core's real edges (gathers row 0;
    # dstloc stays -1 so the indicator kills the contribution).  Counts are
    # therefore identical across cores -> no dynamic trim registers.
    idx_flat = np.zeros((NCORES, tot_slots), np.int16)
    for c in range(NCORES):
        arr = idx_flat[c]
        for p in range(npair):
            for k in range(4):
                p0 = seg_ptr[(c * npair + p) * 4 + k]
                p1 = seg_ptr[(c * npair + p) * 4 + k + 1]
                base = seg_chunk0[(p, k)] * P
                arr[base : base + p1 - p0] = s_bidx[p0:p1].astype(np.int16)

    dstloc_s = np.full((NCORES, P, nslots), -1.0, np.float32)
    seg_id = (s_core * npair + s_pair) * 4 + s_bank
    pos_in_seg = np.arange(e) - seg_ptr[seg_id]
    seg_chunk0_arr = np.zeros(NCORES * npair * 4, np.int64)
    for p in range(npair):
        for k in range(4):
            for c in range(NCORES):
                seg_chunk0_arr[(c * npair + p) * 4 + k] = seg_chunk0[(p, k)]
    chunk_of_edge = seg_chunk0_arr[seg_id] + pos_in_seg // P
    part_of_edge = pos_in_seg % P
    slot_lut = np.full((totc, nblk), -1, np.int64)
    for i, sl in enumerate(slots):
        slot_lut[sl["chunk_gid"], sl["blk"]] = i
    slot_of_edge = slot_lut[chunk_of_edge, s_blk]
    assert (slot_of_edge >= 0).all()
    dstloc_s[s_core, part_of_edge, slot_of_edge] = s_dstloc

    idx_stream = np.ascontiguousarray(
        idx_flat.reshape(NCORES, tot_slots // 16, 16).transpose(0, 2, 1)
    )
    idx_stream = np.tile(idx_stream, (1, 8, 1))

    ls = LayerSched()
    ls.npair = npair
    ls.has_slots = has_slots
    ls.gathers = gathers
    ls.slots = slots
    ls.totc, ls.nslots, ls.tot_slots = totc, nslots, tot_slots
    # max per-(gather, sb) group size -> indicator tile width
    ls.grpmax = max(
        (grp[1] - grp[0] for g in gathers for grp in g["groups"].values()),
        default=1,
    )
    ls.gchmax = max(g["nch"] for g in gathers)
    ls.idx_stream = idx_stream
    ls.idx_flat = idx_flat
    ls.dstloc_s = dstloc_s.astype(ml_dtypes.bfloat16)
    return ls


def _preprocess(n, edge_index):
    src0 = np.asarray(edge_index[0], np.int64)
    dst0 = np.asarray(edge_index[1], np.int64)
    shard = n // NCORES
    nblk = (shard + P - 1) // P
    n_sb = (nblk + SB_N - 1) // SB_N
    qb = _quarter_bounds(nblk)
    qrow_start = [b0 * P for b0, b1 in qb]
    qrows = [min(b1 * P, shard) - b0 * P for b0, b1 in qb]
    bank2_n = [NCORES * r for r in qrows]
    assert all(b <= 32767 for b in bank2_n)
    bank1_rows = (n + 3) // 4  # rank-pair banks for layer 1 (slices of hfull)
    assert bank1_rows <= 32767

    deg = np.bincount(dst0, minlength=n).astype(np.float64) + 1.0
    dinv = (1.0 / np.sqrt(deg)).astype(np.float32)

    # self-loops are NOT in the edge stream: both layers fold them in with
    # one identity matmul per block from the per-core hloc/h2loc tensors.
    src = src0
    dst = dst0

    core_d = dst // shard
    dl = dst - core_d * shard
    blk = dl // P
    dstloc = (dl % P).astype(np.int64)
    sb = blk // SB_N

    # layer-1 bank mapping: contiguous row slices of hfull (rank-major)
    bank_a = src // bank1_rows
    bidx_a = src - bank_a * bank1_rows

    # layer-2 bank mapping: quarter-stacked
    core_s = src // shard
    off = src - core_s * shard
    sblk = off // P
    qb_arr = np.zeros(nblk, np.int64)
    for q, (b0, b1) in enumerate(qb):
        qb_arr[b0:b1] = q
    bank_b = qb_arr[sblk]
    bidx_b = core_s * np.array(qrows)[bank_b] + (off - np.array(qrow_start)[bank_b])

    pair = sb // G_SB
    scheds = []
    for bank, bidx in ((bank_a, bidx_a), (bank_b, bidx_b)):
        order = np.lexsort((blk, bank, pair, core_d))
        scheds.append(
            _make_layer_sched(
                n,
                nblk,
                n_sb,
                core_d[order],
                pair[order],
                bank[order],
                blk[order],
                bidx[order],
                dstloc[order],
            )
        )

    sch = Schedule()
    sch.n, sch.shard, sch.nblk, sch.n_sb = n, shard, nblk, n_sb
    sch.e = src.shape[0]
    sch.qb, sch.qrow_start, sch.qrows = qb, qrow_start, qrows
    sch.bank1_rows, sch.bank2_n = bank1_rows, bank2_n
    sch.dinv = dinv
    sch.L1, sch.L2 = scheds
    sch.grpmax = max(sch.L1.grpmax, sch.L2.grpmax)
    sch.gchmax = max(sch.L1.gchmax, sch.L2.gchmax)
    sch.nidx_vals = sorted(
        {g["nidx"] for g in sch.L1.gathers} | {g["nidx"] for g in sch.L2.gathers}
    )
    return sch


# ----------------------------------------------------------------- build ---
def _build(sch, in_dim, hid, out_dim):
    import concourse.mybir as mybir
    import concourse.tile as tile
    from concourse import bacc

    bf16 = mybir.dt.bfloat16
    f32 = mybir.dt.float32
    shard, nblk, n_sb = sch.shard, sch.nblk, sch.n_sb
    grpmax = sch.grpmax
    gchmax = sch.gchmax
    qb = sch.qb
    n = sch.n

    nc = bacc.Bacc(num_swdge_queues=NQUEUES)

    xT = nc.declare_dram_parameter("xT", [in_dim, shard], bf16, isOutput=False)
    idxs1 = nc.declare_dram_parameter(
        "idxs1", [P, sch.L1.tot_slots // 16], mybir.dt.int16, isOutput=False
    )
    idxs2 = nc.declare_dram_parameter(
        "idxs2", [P, sch.L2.tot_slots // 16], mybir.dt.int16, isOutput=False
    )
    dstloc1 = nc.declare_dram_parameter(
        "dstloc1", [P, sch.L1.nslots], bf16, isOutput=False
    )
    dstloc2 = nc.declare_dram_parameter(
        "dstloc2", [P, sch.L2.nslots], bf16, isOutput=False
    )
    iotar_in = nc.declare_dram_parameter("iotar", [P, grpmax * P], bf16, isOutput=False)
    dinvbc = nc.declare_dram_parameter("dinvbc", [P, nblk * P], f32, isOutput=False)
    dinvb = nc.declare_dram_parameter("dinvb", [P, nblk], f32, isOutput=False)
    w1 = nc.declare_dram_parameter("W1", [in_dim, hid], bf16, isOutput=False)
    b1 = nc.declare_dram_parameter("b1", [hid, 1], f32, isOutput=False)
    w2 = nc.declare_dram_parameter("W2", [hid, out_dim], bf16, isOutput=False)
    b2bc = nc.declare_dram_parameter("b2bc", [P, out_dim], f32, isOutput=False)
    ident_in = nc.declare_dram_parameter("ident", [P, P], bf16, isOutput=False)
    out_ext = nc.declare_dram_parameter("out", [shard, out_dim], f32, isOutput=True)

    dummy_tbl = nc.dram_tensor("dummy_tbl", [1, P], bf16)
    hloc = nc.dram_tensor("hloc", [shard, P], bf16)
    hfull = nc.dram_tensor("hfull", [n, P], bf16, addr_space="Shared")
    h2loc_q = [
        nc.dram_tensor(f"h2loc{q}", [sch.qrows[q], P], bf16) for q in range(4)
    ]
    h2bank = [
        nc.dram_tensor(f"h2bank{q}", [sch.bank2_n[q], P], bf16, addr_space="Shared")
        for q in range(4)
    ]

    kin = in_dim // P

    def quarter_of(b):
        for q, (b0, b1) in enumerate(qb):
            if b0 <= b < b1:
                return q
        raise AssertionError

    def layer_maps(ls):
        seg_gathers = {}
        for g in ls.gathers:
            seg_gathers.setdefault((g["pair"], g["bank"]), []).append(g)
        return seg_gathers

    seg1 = layer_maps(sch.L1)
    seg2 = layer_maps(sch.L2)

    ag2_at_sb = {}
    for q in range(4):
        sq_end = (qb[q][1] - 1) // SB_N
        key = min(sq_end + AG2_LAG, n_sb - 1) if q < 3 else n_sb - 1
        ag2_at_sb.setdefault(key, []).append(q)

    with tile.TileContext(nc) as tc:
        with (
            tc.tile_pool(name="const", bufs=1) as cpool,
            tc.tile_pool(name="xload", bufs=2) as xpool,
            tc.tile_pool(name="hb", bufs=3) as hbpool,
            tc.tile_pool(name="idx", bufs=16) as ipool,
            tc.tile_pool(name="gath", bufs=GBUFS) as gpool,
            tc.tile_pool(name="prep", bufs=max(4 * PREP_SB, 1)) as ppool,
            tc.tile_pool(name="sind", bufs=SBUFS) as spool,
            tc.tile_pool(name="dvp", bufs=3) as dvpool,
            tc.tile_pool(name="rl", bufs=8) as rpool,
            tc.tile_pool(name="blk", bufs=3) as bpool,
            tc.tile_pool(name="ob", bufs=2) as opool,
            tc.tile_pool(name="psh", bufs=1, space="PSUM") as psh,
            tc.tile_pool(
                name="psagg", bufs=(L2_K3_DELAY + 1) * SB_N, space="PSUM"
            ) as psagg,
            tc.tile_pool(name="psh2", bufs=1, space="PSUM") as psh2,
        ):
            import contextlib

            regstack = contextlib.ExitStack()
            # one pinned register per distinct static gather size.  Each is
            # read once into a scratch register immediately, so Tile places
            # every MOVE at t~0 -- otherwise a register first used late (e.g.
            # only by layer 2) gets its MOVE scheduled just before that use
            # with a conservative WAR wait on all prior collectives.
            nidx_regs = {}
            scratch_reg = regstack.enter_context(nc.gpsimd.register("nidx_scr"))
            for v in sch.nidx_vals:
                r = regstack.enter_context(nc.gpsimd.register(f"nidx_{v}"))
                nc.gpsimd.reg_mov(r, v)
                nidx_regs[v] = r
            for v in sch.nidx_vals:
                nc.gpsimd.reg_mov(scratch_reg, nidx_regs[v])

            # ---- constants into SBUF
            w1_t = [
                cpool.tile([P, hid], bf16, tag=f"w1_{k}", name=f"w1t{k}")
                for k in range(kin)
            ]
            for k in range(kin):
                nc.sync.dma_start(out=w1_t[k][:], in_=w1[k * P : (k + 1) * P, :])
            w2_sb = cpool.tile([hid, out_dim], bf16, tag="w2")
            nc.sync.dma_start(out=w2_sb[:], in_=w2[:])
            b1_sb = cpool.tile([hid, 1], f32, tag="b1")
            nc.sync.dma_start(out=b1_sb[:], in_=b1[:])
            b2_sb = cpool.tile([P, out_dim], f32, tag="b2")
            nc.sync.dma_start(out=b2_sb[:], in_=b2bc[:])
            dinvb_sb = cpool.tile([P, nblk], f32, tag="dinvb")
            nc.sync.dma_start(out=dinvb_sb[:], in_=dinvb[:])
            dstloc1_sb = cpool.tile([P, sch.L1.nslots], bf16, tag="dstloc1")
            nc.sync.dma_start(out=dstloc1_sb[:], in_=dstloc1[:])
            dstloc2_sb = cpool.tile([P, sch.L2.nslots], bf16, tag="dstloc2")
            nc.sync.dma_start(out=dstloc2_sb[:], in_=dstloc2[:])
            iotar_sb = cpool.tile([P, grpmax * P], bf16, tag="iotar")
            nc.sync.dma_start(out=iotar_sb[:], in_=iotar_in[:])
            ident_sb = cpool.tile([P, P], bf16, tag="ident")
            nc.sync.dma_start(out=ident_sb[:], in_=ident_in[:])

            def bank_table(layer, k):
                if layer == 1:
                    r0 = k * sch.bank1_rows
                    r1 = min(r0 + sch.bank1_rows, n)
                    return hfull[r0:r1, :]
                return h2bank[k][0 : sch.bank2_n[k], :]

            def issue_gather(layer, g, it, prep_sem=None):
                nidx, nch = g["nidx"], g["nch"]
                pool = gpool if prep_sem is None else ppool
                gt = pool.tile([P, gchmax, P], bf16, tag="gt", name="gt")
                nc.gpsimd.dma_gather(
                    out_ap=gt[:, :nch, :],
                    in_ap=bank_table(layer, g["bank"]),
                    idxs_ap=it[:, : nidx // 16],
                    num_idxs=nidx,
                    num_idxs_reg=nidx_regs[nidx],
                    elem_size=P,
                    single_packet=False,
                    queue_num=g["bank"],
                    prepare_only=prep_sem is not None,
                    sem=prep_sem,
                )
                return gt

            def load_it(layer, g):
                idxs = idxs1 if layer == 1 else idxs2
                it = ipool.tile([P, gchmax * 8], mybir.dt.int16, tag="it")
                nc.sync.dma_start(
                    out=it[:, : g["nidx"] // 16],
                    in_=idxs[:, g["c16"] : g["c16"] + g["nidx"] // 16],
                )
                return it

            # ---- leading L1 gathers: desc-gen during the prefix via
            # prepare_only; DMA fires via trigger_dma once AG1 lands (the
            # table RAW is deferred to the trigger).  Half before the AG1
            # trigger instruction (Q7 idle then), half after (desc-gen runs
            # while the collective is in flight).
            prep_sems = [
                nc.alloc_semaphore(f"prepq{q}") for q in range(NQUEUES)
            ]
            preissued = {}
            prep_target = {}  # gi -> (bank, cumulative dma-sem value)
            prep_count = [0] * NQUEUES

            def prep_wave(s_lo, s_hi):
                for s in range(s_lo, min(s_hi, n_sb)):
                    for k in range(4):
                        for g in seg1.get((s, k), []):
                            it = load_it(1, g)
                            preissued[g["gi"]] = issue_gather(
                                1, g, it, prep_sem=prep_sems[g["bank"]]
                            )
                            prep_count[g["bank"]] += 1
                            prep_target[g["gi"]] = (
                                g["bank"],
                                16 * prep_count[g["bank"]],
                            )

            prep_wave(0, PREP_SB // 2)

            # ---- h~ = (dinv*x) @ W1, shard-local (x pre-scaled by dinv);
            # 4 blocks batched per PSUM bank, double-buffered
            for g0 in range(0, nblk, XGRP):
                g1 = min(g0 + XGRP, nblk)
                c0, c1 = g0 * P, min(g1 * P, shard)
                xt = [
                    xpool.tile([P, XGRP * P], bf16, tag=f"xt{k}", name=f"xt{k}")
                    for k in range(kin)
                ]
                for k in range(kin):
                    nc.sync.dma_start(
                        out=xt[k][:, : c1 - c0], in_=xT[k * P : (k + 1) * P, c0:c1]
                    )
                for s0 in range(g0, g1, 4):
                    s1 = min(s0 + 4, g1)
                    hp = psh.tile([P, 4 * hid], f32, tag="hps")
                    mtot = min(s1 * P, shard) - s0 * P
                    for b in range(s0, s1):
                        m = min(P, shard - b * P)
                        sub = b - s0
                        for k in range(kin):
                            nc.tensor.matmul(
                                out=hp[:m, sub * hid : (sub + 1) * hid],
                                lhsT=xt[k][:, b * P - c0 : b * P - c0 + m],
                                rhs=w1_t[k][:],
                                start=(k == 0),
                                stop=(k == kin - 1),
                            )
                    nsub = s1 - s0
                    hsb = hbpool.tile([P, 4, hid], bf16, tag="hsb")
                    nc.scalar.activation(
                        out=hsb[:, :nsub, :],
                        in_=hp[:, : nsub * hid].rearrange("p (g f) -> p g f", g=nsub),
                        func=mybir.ActivationFunctionType.Copy,
                    )
                    nfull = mtot // P
                    if nfull:
                        nc.sync.dma_start(
                            out=hloc[s0 * P : s0 * P + nfull * P, :].rearrange(
                                "(g p) f -> p g f", p=P
                            ),
                            in_=hsb[:, :nfull, :],
                        )
                    if mtot > nfull * P:
                        mp = mtot - nfull * P
                        nc.sync.dma_start(
                            out=hloc[s0 * P + nfull * P : s0 * P + mtot, :],
                            in_=hsb[:mp, nfull, :],
                        )

            nc.gpsimd.collective_compute(
                "AllGather",
                mybir.AluOpType.bypass,
                ins=[hloc[:]],
                outs=[hfull[:]],
                replica_groups=[list(range(NCORES))],
            )

            prep_wave(PREP_SB // 2, PREP_SB)
            if PREP_SB > 0:
                for q in range(NQUEUES):
                    nc.gpsimd.trigger_dma(count=None, queue_num=q)

            def build_ind(grp_start, grp_end, dstloc_sb):
                nsl = grp_end - grp_start
                sbig = spool.tile([P, grpmax, P], bf16, tag="sind")
                nc.vector.tensor_tensor(
                    out=sbig[:, :nsl, :],
                    in0=iotar_sb[:, : nsl * P].rearrange("p (k f) -> p k f", k=nsl),
                    in1=dstloc_sb[:, grp_start:grp_end].to_broadcast([P, nsl, P]),
                    op=mybir.AluOpType.is_equal,
                )
                return sbig

            def run_layer(layer):
                """Gathers are merged per (super-block pair, bank); slot
                matmuls and epilogues stay per super-block.  Layer 2 delays
                bank-3 gathers by one step so the in-order GpSimd stream
                never parks on the last AG2."""
                seg_gathers = seg1 if layer == 1 else seg2
                ls = sch.L1 if layer == 1 else sch.L2
                dstloc_sb = dstloc1_sb if layer == 1 else dstloc2_sb
                delay = 0 if layer == 1 else L2_K3_DELAY
                w = P if layer == 1 else out_dim

                def issued_at(t):
                    """gathers whose dma_gather is issued at step t."""
                    out = []
                    if t < n_sb and t % G_SB == 0:
                        p = t // G_SB
                        ks = range(4) if layer == 1 else range(3)
                        for k in ks:
                            out.extend(seg_gathers.get((p, k), []))
                    if layer == 2 and 0 <= t - 1 < n_sb and (t - 1) % G_SB == 0:
                        p = (t - 1) // G_SB
                        out.extend(seg_gathers.get((p, 3), []))
                    return out

                def slot_groups_at(t):
                    """(gather, sb) slot groups whose matmuls run at step t."""
                    out = []
                    if t < n_sb:
                        p = t // G_SB
                        ks = range(4) if layer == 1 else range(3)
                        for k in ks:
                            for g in seg_gathers.get((p, k), []):
                                if t in g["groups"]:
                                    out.append((g, t))
                    if layer == 2 and 0 <= t - 1 < n_sb:
                        p = (t - 1) // G_SB
                        for g in seg_gathers.get((p, 3), []):
                            if t - 1 in g["groups"]:
                                out.append((g, t - 1))
                    return out

                def prefetch_it(t):
                    return {
                        g["gi"]: load_it(layer, g)
                        for g in issued_at(t)
                        if g["gi"] not in preissued
                    }

                def prefetch_ind(t):
                    tiles = {}
                    for g, s in slot_groups_at(t):
                        grp = g["groups"][s]
                        tiles[(g["gi"], s)] = build_ind(grp[0], grp[1], dstloc_sb)
                    return tiles

                def prefetch_rl(s):
                    tiles = {}
                    if not (0 <= s < n_sb):
                        return tiles
                    blocks = list(range(s * SB_N, min((s + 1) * SB_N, nblk)))
                    # batch contiguous full blocks within one source tensor
                    runs = []
                    for b in blocks:
                        q = quarter_of(b) if layer == 2 else 0
                        if runs and runs[-1][2] == q and runs[-1][1] == b:
                            runs[-1][1] = b + 1
                        else:
                            runs.append([b, b + 1, q])
                    for b0, b1, q in runs:
                        nb = b1 - b0
                        m_end = min(b1 * P, shard) - b0 * P
                        rl = rpool.tile([P, SB_N, P], bf16, tag="rl")
                        src = hloc if layer == 1 else h2loc_q[q]
                        r0 = b0 * P - (0 if layer == 1 else sch.qrow_start[q])
                        nfull = m_end // P
                        if nfull:
                            nc.sync.dma_start(
                                out=rl[:, :nfull, :],
                                in_=src[r0 : r0 + nfull * P, :].rearrange(
                                    "(g p) f -> p g f", p=P
                                ),
                            )
                        if m_end > nfull * P:
                            mp = m_end - nfull * P
                            nc.sync.dma_start(
                                out=rl[:mp, nfull, :],
                                in_=src[r0 + nfull * P : r0 + m_end, :],
                            )
                        for b in range(b0, b1):
                            tiles[b] = (rl, b - b0)
                    return tiles

                def prefetch_dv(s):
                    if layer != 1 or not (0 <= s < n_sb):
                        return None
                    b0 = s * SB_N
                    b1 = min(b0 + SB_N, nblk)
                    dv = dvpool.tile([P, SB_N * P], f32, tag="dv")
                    nc.sync.dma_start(
                        out=dv[:, : (b1 - b0) * P], in_=dinvbc[:, b0 * P : b1 * P]
                    )
                    return dv

                it_tiles = prefetch_it(0)
                ind_tiles = prefetch_ind(0)
                rl_tiles = prefetch_rl(0)
                dv_cur = prefetch_dv(0)
                rl_hold = {}
                dv_hold = {}
                agg_hold = {}
                gt_tiles = {}
                nsteps = n_sb + delay
                for t in range(nsteps):
                    s_new = t if t < n_sb else None
                    s_old = t - delay if 0 <= t - delay < n_sb else None
                    next_it = prefetch_it(t + 1)
                    next_ind = prefetch_ind(t + 1)
                    next_rl = prefetch_rl(t + 1)
                    next_dv = prefetch_dv(t + 1)

                    if s_new is not None:
                        blocks = list(
                            range(s_new * SB_N, min((s_new + 1) * SB_N, nblk))
                        )
                        agg_t = {
                            b: psagg.tile(
                                [P, w], f32, tag="agg", name=f"agg{layer}_{s_new}_{b}"
                            )
                            for b in blocks
                        }
                        agg_hold[s_new] = agg_t
                        rl_hold[s_new] = rl_tiles
                        dv_hold[s_new] = dv_cur
                        # self-loop contribution opens each block's PSUM group
                        for b in blocks:
                            m = min(P, shard - b * P)
                            rl, sub = rl_tiles[b]
                            solo = (s_new, b) not in ls.has_slots
                            if layer == 1:
                                nc.tensor.matmul(
                                    out=agg_t[b][:, :],
                                    lhsT=rl[:m, sub, :],
                                    rhs=ident_sb[:m, :],
                                    start=True,
                                    stop=solo,
                                )
                            else:
                                nc.tensor.matmul(
                                    out=agg_t[b][:, :],
                                    lhsT=ident_sb[:m, :],
                                    rhs=rl[:m, sub, :out_dim],
                                    start=True,
                                    stop=solo,
                                )

                    for g in issued_at(t):
                        if layer == 1 and g["gi"] in preissued:
                            gt_tiles[g["gi"]] = preissued.pop(g["gi"])
                        else:
                            gt_tiles[g["gi"]] = issue_gather(
                                layer, g, it_tiles[g["gi"]]
                            )

                    for g, s in slot_groups_at(t):
                        gt = gt_tiles[g["gi"]]
                        sbig = ind_tiles[(g["gi"], s)]
                        agg_t = agg_hold[s]
                        if layer == 1 and g["gi"] in prep_target:
                            # explicit PE-side wait on the prep's DMA sem --
                            # Tile's lane model does not cover manual
                            # prepare_only semaphores.
                            bk, tgt = prep_target.pop(g["gi"])
                            nc.tensor.wait_ge(prep_sems[bk], tgt)
                        for sl in g["groups"][s][2]:
                            if layer == 1:
                                nc.tensor.matmul(
                                    out=agg_t[sl["blk"]][:, :],
                                    lhsT=gt[:, sl["cl"], :],
                                    rhs=sbig[:, sl["sl_in_grp"], :],
                                    start=sl["start"],
                                    stop=sl["stop"],
                                )
                            else:
                                nc.tensor.matmul(
                                    out=agg_t[sl["blk"]][:, :],
                                    lhsT=sbig[:, sl["sl_in_grp"], :],
                                    rhs=gt[:, sl["cl"], :out_dim],
                                    start=sl["start"],
                                    stop=sl["stop"],
                                )

                    # ---- block epilogues for the super-block closing now
                    if s_old is not None:
                        blocks = list(
                            range(s_old * SB_N, min((s_old + 1) * SB_N, nblk))
                        )
                        agg_t = agg_hold.pop(s_old)
                        dv_sb = dv_hold.pop(s_old)
                        rl_hold.pop(s_old)
                        ob = (
                            opool.tile([P, SB_N, out_dim], f32, tag="ob", name="ob")
                            if layer == 2
                            else None
                        )
                        for b in blocks:
                            m = min(P, shard - b * P)
                            off = (b - s_old * SB_N) * P
                            if layer == 1:
                                t1 = bpool.tile([P, P], bf16, tag="t1")
                                nc.vector.tensor_tensor(
                                    out=t1[:],
                                    in0=agg_t[b][:, :],
                                    in1=dv_sb[:, off : off + P],
                                    op=mybir.AluOpType.mult,
                                )
                                o1 = bpool.tile([P, P], bf16, tag="o1")
                                nc.scalar.activation(
                                    out=o1[:],
                                    in_=t1[:],
                                    func=mybir.ActivationFunctionType.Relu,
                                    bias=b1_sb[:, :1],
                                )
                                h2p = psh2.tile([P, out_dim], f32, tag="h2p")
                                nc.tensor.matmul(
                                    out=h2p[:],
                                    lhsT=o1[:],
                                    rhs=w2_sb[:],
                                    start=True,
                                    stop=True,
                                )
                                h2s = bpool.tile([P, P], bf16, tag="h2s")
                                nc.scalar.activation(
                                    out=h2s[:m, :out_dim],
                                    in_=h2p[:m, :],
                                    func=mybir.ActivationFunctionType.Copy,
                                    scale=dinvb_sb[:m, b : b + 1],
                                )
                                q = quarter_of(b)
                                r0 = b * P - sch.qrow_start[q]
                                nc.sync.dma_start(
                                    out=h2loc_q[q][r0 : r0 + m, 0:out_dim],
                                    in_=h2s[:m, :out_dim],
                                )
                            else:
                                sub = b - blocks[0]
                                t2 = bpool.tile([P, out_dim], f32, tag="t2")
                                nc.scalar.activation(
                                    out=t2[:m, :],
                                    in_=agg_t[b][:m, :],
                                    func=mybir.ActivationFunctionType.Copy,
                                    scale=dinvb_sb[:m, b : b + 1],
                                )
                                nc.vector.tensor_tensor(
                                    out=ob[:m, sub, :],
                                    in0=t2[:m, :],
                                    in1=b2_sb[:m, :],
                                    op=mybir.AluOpType.add,
                                )
                        if layer == 2:
                            b0 = blocks[0]
                            m_end = min(blocks[-1] * P + P, shard) - b0 * P
                            nfull = m_end // P
                            if nfull:
                                nc.sync.dma_start(
                                    out=out_ext[
                                        b0 * P : b0 * P + nfull * P, :
                                    ].rearrange("(g p) f -> p g f", p=P),
                                    in_=ob[:, :nfull, :],
                                )
                            if m_end > nfull * P:
                                mp = m_end - nfull * P
                                nc.sync.dma_start(
                                    out=out_ext[b0 * P + nfull * P : b0 * P + m_end, :],
                                    in_=ob[:mp, nfull, :],
                                )

                    if layer == 1 and s_old is not None:
                        for q in ag2_at_sb.get(s_old, []):
                            nc.gpsimd.collective_compute(
                                "AllGather",
                                mybir.AluOpType.bypass,
                                ins=[h2loc_q[q][:]],
                                outs=[h2bank[q][:]],
                                replica_groups=[list(range(NCORES))],
                            )

                    it_tiles = next_it
                    ind_tiles = next_ind
                    rl_tiles = next_rl
                    dv_cur = next_dv

            run_layer(1)
            run_layer(2)
            regstack.close()

    nc.compile()
    return nc


# ---------------------------------------------------------------- kernel ---
def _make_in_maps(sch, x, W1, b1v, W2, b2v):
    hid = W1.shape[1]
    out_dim = W2.shape[1]
    shard, nblk = sch.shard, sch.nblk
    bf = ml_dtypes.bfloat16
    in_maps = []
    w1b = W1.astype(bf)
    w2b = W2.astype(bf)
    b1c = b1v.reshape(hid, 1).astype(np.float32).copy()
    b2c = np.broadcast_to(b2v.astype(np.float32), (P, out_dim)).copy()
    iotar = np.tile(np.arange(P, dtype=np.float32), (P, sch.grpmax)).astype(bf)
    ident = np.eye(P, dtype=np.float32).astype(bf)
    xs_all = (x * sch.dinv[:, None]).astype(bf)
    for c in range(NCORES):
        xs = np.ascontiguousarray(xs_all[c * shard : (c + 1) * shard].T)
        dv = sch.dinv[c * shard : (c + 1) * shard]
        full = np.zeros(nblk * P, np.float32)
        full[:shard] = dv
        dvb = np.ascontiguousarray(full.reshape(nblk, P).T)
        dbc = np.broadcast_to(full, (P, nblk * P)).copy()
        in_maps.append(
            {
                "xT": xs,
                "idxs1": sch.L1.idx_stream[c],
                "idxs2": sch.L2.idx_stream[c],
                "dstloc1": sch.L1.dstloc_s[c],
                "dstloc2": sch.L2.dstloc_s[c],
                "dinvb": dvb,
                "W1": w1b,
                "b1": b1c,
                "W2": w2b,
                "b2bc": b2c,
                "iotar": iotar,
                "ident": ident,
                "dinvbc": dbc,
            }
        )
    return in_maps


def _get_compiled(n, e, edge_index, in_dim, hid, out_dim):
    key = ("nc", n, e)
    if key not in _CACHE:
        sch = _preprocess(n, edge_index)
        _CACHE[("sched", n, e)] = sch
        _CACHE[key] = _build(sch, in_dim, hid, out_dim)
    return _CACHE[("sched", n, e)], _CACHE[key]


def kernel(x, edge_index, W1, b1, W2, b2):
    _install_compat()
    from concourse.bass_utils import run_bass_kernel_spmd

    x = np.asarray(x)
    edge_index = np.asarray(edge_index)
    W1 = np.asarray(W1, np.float32)
    b1v = np.asarray(b1, np.float32)
    W2 = np.asarray(W2, np.float32)
    b2v = np.asarray(b2, np.float32)
    n, in_dim = x.shape
    hid = W1.shape[1]
    out_dim = W2.shape[1]

    sch, nc = _get_compiled(n, edge_index.shape[1], edge_index, in_dim, hid, out_dim)
    in_maps = _make_in_maps(sch, x, W1, b1v, W2, b2v)
    import os

    trace = bool(os.environ.get("GCN_TRACE"))
    res = run_bass_kernel_spmd(
        nc, in_maps, core_ids=list(range(NCORES)), trace=trace
    )
    global LAST_EXEC_NS
    LAST_EXEC_NS = res.exec_time_ns
    return np.concatenate([res.results[c]["out"] for c in range(NCORES)], axis=0)


LAST_EXEC_NS = None
